# revision 19
# baseline (speedup 1.0000x reference)
"""CondConv2d on 8 Trainium2 NeuronCores — data-parallel over batch N=8.

Per-core (one sample):
  - attention branch collapsed to 10 "basis" sums of x (total / edge rows /
    edge cols / corners / const) contracted with host-precomputed coefficients
    -> logits -> softmax -> per-sample mixed 3x3 weight, with the static
    residual conv fused in (mw = sum_k att_k W_k + conv_w; bias added at
    PSUM eviction).
  - 3x3 conv as 6 accumulating PE matmuls per PSUM tile over a 130-wide
    zero-padded layout; contraction 128 = 64 channels (lower partitions) +
    64 channels of a row-shifted copy (upper partitions), pairing taps
    (-1,w)+(0,w) per matmul.
"""
import os
import numpy as np

N, C, H, W = 8, 64, 128, 128
K = 4
WP = W + 2                 # padded row width (130)
NELEM = WP * WP + 2        # per-partition x buffer length (16902); xpadflat at elem offset 1
ROWS_PER_TILE = 3          # output rows per PSUM tile (free dim 390 <= 512)
NCHUNKS = 4                # x load/reduce chunks (128 rows / NCHUNKS each)

CONV_DT = os.environ.get("KCONV_DT", "fp32r")   # "fp32" | "fp32r" | "bf16"

MM_TAPS = [((-1, -1), (0, -1)), ((-1, 0), (0, 0)), ((-1, 1), (0, 1)),
           ((1, -1), None), ((1, 0), None), ((1, 1), None)]
MM_OFFS = [130 * L[0] + L[1] for L, _ in MM_TAPS]


# ----------------------------------------------------------------------------
# host-side prep
# ----------------------------------------------------------------------------
def _make_cw2(net0_w, net0_b, net1_w, net1_b, net2_w, net2_b):
    """CW2[c, b, k]: logits[k] = sum_{c,b} CW2[c,b,k] * basis[c,b].
    basis: 0=total, 1=row0, 2=row127, 3=col0, 4=col127,
           5..8=corners (00,0W,H0,HW), 9=const 1."""
    cw = np.zeros((C, 10, K), np.float64)
    scale = 1.0 / (C * H * W)
    for w_net, pads in ((net0_w, (0, 0, 0)), (net1_w, (1, 1, 1)), (net2_w, (2, 1, 1))):
        Kk, _, kd, kh, kw = w_net.shape
        pd, ph, pw = pads
        for i in range(kd):
            clo, chi = max(0, i - pd), min(C - 1, C - 1 + i - pd)
            cmask = np.zeros(C)
            cmask[clo:chi + 1] = 1.0
            for j in range(kh):
                hlo, hhi = max(0, j - ph), min(H - 1, H - 1 + j - ph)
                dropA = 0 if hlo == 1 else (127 if hhi == H - 2 else None)
                for l in range(kw):
                    wlo, whi = max(0, l - pw), min(W - 1, W - 1 + l - pw)
                    dropB = 0 if wlo == 1 else (127 if whi == W - 2 else None)
                    v = np.zeros(10)
                    v[0] = 1.0
                    if dropA == 0: v[1] = -1.0
                    if dropA == 127: v[2] = -1.0
                    if dropB == 0: v[3] = -1.0
                    if dropB == 127: v[4] = -1.0
                    if dropA is not None and dropB is not None:
                        v[{(0, 0): 5, (0, 127): 6, (127, 0): 7, (127, 127): 8}[(dropA, dropB)]] = 1.0
                    for k in range(Kk):
                        cw[:, :, k] += w_net[k, 0, i, j, l] * scale * np.outer(cmask, v)
    btot = (net0_b + net1_b + net2_b).astype(np.float64)
    cw[:, 9, :] += btot[None, :] / C
    return np.ascontiguousarray(cw.astype(np.float32))


def _make_bank(Wt):
    """Wt (co, ci, 3, 3) -> (128, 6, 64): [p=ci(lo)/64+ci(hi), mm, co]."""
    bank = np.zeros((128, 6, 64), np.float32)
    for m, (L, Hh) in enumerate(MM_TAPS):
        bank[:64, m, :] = Wt[:, :, 1 + L[0], 1 + L[1]].T
        if Hh is not None:
            bank[64:, m, :] = Wt[:, :, 1 + Hh[0], 1 + Hh[1]].T
    return bank


# ----------------------------------------------------------------------------
# device program
# ----------------------------------------------------------------------------
_NC_CACHE = {}


def _build_nc(conv_dt, skip_attn=False, skip_conv=False, skip_logits_mm=False,
              spans_mode="mixed", with_edges=True, with_cols=True, with_corners=True):
    import concourse.bacc as bacc
    import concourse.tile as tile
    from concourse import mybir

    f32 = mybir.dt.float32
    if conv_dt == "bf16":
        DT = mybir.dt.bfloat16
    elif conv_dt == "fp32r":
        DT = mybir.dt.float32r
    else:
        DT = f32
    Alu = mybir.AluOpType
    Ax = mybir.AxisListType
    Act = mybir.ActivationFunctionType

    nc = bacc.Bacc("TRN2", target_bir_lowering=False, debug=False,
                   enable_asserts=False, num_devices=N)
    xin = nc.dram_tensor("xin", [C, H * W], DT, kind="ExternalInput")
    wbk = nc.dram_tensor("wbanks", [128, 5, 6 * 64], f32, kind="ExternalInput")
    cw2 = nc.dram_tensor("cw2", [C, 10, K], f32, kind="ExternalInput")
    cb = nc.dram_tensor("convb", [C, 1], f32, kind="ExternalInput")
    outT = nc.dram_tensor("out", [C, H, W], f32, kind="ExternalOutput")

    rows_per_chunk = H // NCHUNKS                      # 32
    span_elems = WP * rows_per_chunk                   # 4160

    with tile.TileContext(nc) as tc:
        with tc.tile_pool(name="singles", bufs=1) as S, \
             tc.tile_pool(name="stage", bufs=4) as STG, \
             tc.tile_pool(name="cpsum", bufs=4, space="PSUM") as PS, \
             tc.tile_pool(name="spsum", bufs=1, space="PSUM") as PS1:

            XL = S.tile([128, NELEM], DT)
            wb_sb = S.tile([128, 5, 6 * 64], f32)
            cw2_sb = S.tile([C, 10, K], f32)
            convb_sb = S.tile([C, 1], f32)
            onesrow = S.tile([128, 128], f32)
            ones1 = S.tile([1, 128], f32)
            att_sb = S.tile([128, K], f32)
            attbc = S.tile([128, K], f32)
            M10 = S.tile([C, 10], f32)
            PART = S.tile([C, NCHUNKS], f32)
            mw = None  # allocated below once MWDT is known
            mwb = S.tile([128, 6, 64], DT, name="mwb") if conv_dt == "bf16" else None
            fold = S.tile([C, 2600], f32)
            actout = S.tile([C, 3300], f32)
            mx = S.tile([1, 1], f32)
            mxn = S.tile([1, 1], f32)
            rs = S.tile([1, 1], f32)

            wpsum = PS1.tile([128, 128], f32)
            psum_l = PS1.tile([1, K], f32)
            psum_b = PS1.tile([128, K], f32)

            XLv = XL.bitcast(f32) if conv_dt == "fp32r" else XL
            MWDT = mybir.dt.float32r if conv_dt == "fp32r" else f32
            mw = S.tile([128, 6, 64], MWDT, name="mw")
            # --- constants / border zeroing (DVE, all tiny) ---
            nc.vector.memset(onesrow, 0.0)
            nc.vector.memset(att_sb, 0.0)
            nc.vector.memset(ones1, 1.0)
            nc.vector.memset(M10[:, 9:10], 1.0)
            # lower borders: head, row gaps (2 elems each), tail
            nc.vector.memset(XLv[0:64, 0:132], 0.0)
            nc.vector.memset(
                XLv[0:64, 260:260 + WP * 127].rearrange("p (r w) -> p r w", w=WP)[:, :, 0:2], 0.0)
            nc.vector.memset(XLv[0:64, 16770:NELEM], 0.0)
            # upper borders
            nc.vector.memset(XLv[64:128, 0:2], 0.0)
            nc.vector.memset(
                XLv[64:128, WP:WP + WP * 127].rearrange("p (r w) -> p r w", w=WP)[:, :, 0:2], 0.0)
            nc.vector.memset(XLv[64:128, 16640:NELEM], 0.0)

            # --- small input DMAs ---
            nc.sync.dma_start(out=wb_sb, in_=wbk[:, :, :])
            nc.sync.dma_start(out=cw2_sb, in_=cw2[:, :, :])
            nc.sync.dma_start(out=convb_sb, in_=cb[:, :])

            # --- x load (lower: offset 132; upper row-shifted copy: offset 2) ---
            xsrc = xin[:, :].rearrange("p (r w) -> p r w", w=W)
            for c in range(NCHUNKS):
                r0 = c * rows_per_chunk
                dst_lo = XL[0:64, 132 + span_elems * c: 132 + span_elems * (c + 1)] \
                    .rearrange("p (r w) -> p r w", w=WP)[:, :, 0:W]
                nc.sync.dma_start(out=dst_lo, in_=xsrc[:, r0:r0 + rows_per_chunk, :])
                dst_hi = XL[64:128, 2 + span_elems * c: 2 + span_elems * (c + 1)] \
                    .rearrange("p (r w) -> p r w", w=WP)[:, :, 0:W]
                nc.sync.dma_start(out=dst_hi, in_=xsrc[:, r0:r0 + rows_per_chunk, :])

            # --- PE warm-up (results discarded; onesrow is all-zero here) ---
            for i in range(4):
                nc.tensor.matmul(wpsum, onesrow, onesrow, start=True, stop=True)
            for c in range(NCHUNKS):
                a = 132 + span_elems * c
                for i in range(2):
                    # both operands from XL so the warm-up rides the chunk-DMA deps
                    nc.tensor.matmul(wpsum, XL[:, a:a + 128], XL[:, a + 128 * i:a + 128 * i + 128],
                                     start=True, stop=True)

            # --- attention basis sums ---
            # DVE: scalar_tensor_tensor fold (reads 2 streams/cycle) with accum_out;
            # ACT: activation-Identity with accum_out.  (tensor_tensor_reduce is
            # broken on this runtime — do not use.)
            spans = [(0, 5200, "dve"), (5200, 5200, "dve"),
                     (10400, 3250, "act"), (13650, NELEM - 13650, "act")]
            if skip_attn:
                spans = []
            for idx, (a, ln, eng) in enumerate(spans):
                if spans_mode == "basic":
                    eng = "basic"
                elif spans_mode == "act":
                    eng = "act"
                if eng == "dve":
                    h = ln // 2
                    nc.vector.scalar_tensor_tensor(
                        out=fold[:, :h], in0=XLv[0:64, a:a + h], scalar=1.0,
                        in1=XLv[0:64, a + h:a + ln], op0=Alu.mult, op1=Alu.add,
                        accum_out=PART[:, idx:idx + 1])
                elif eng == "act":
                    nc.scalar.activation(
                        out=actout[:, :ln], in_=XLv[0:64, a:a + ln], func=Act.Identity,
                        bias=0.0, scale=1.0, accum_out=PART[:, idx:idx + 1])
                else:
                    nc.vector.tensor_reduce(out=PART[:, idx:idx + 1], in_=XLv[0:64, a:a + ln],
                                            axis=Ax.X, op=Alu.add)
            if not skip_attn:
                nc.vector.tensor_reduce(out=M10[:, 0:1], in_=PART, axis=Ax.X, op=Alu.add)
                if with_edges:
                    nc.vector.tensor_reduce(out=M10[:, 1:2], in_=XLv[0:64, 132:132 + W], axis=Ax.X, op=Alu.add)
                    nc.vector.tensor_reduce(out=M10[:, 2:3], in_=XLv[0:64, 16642:16642 + W], axis=Ax.X, op=Alu.add)
                else:
                    nc.vector.memset(M10[:, 1:3], 0.0)
                if with_cols:
                    col0 = XLv[0:64, 132:132 + WP * H].rearrange("p (r w) -> p r w", w=WP)[:, :, 0:1]
                    nc.vector.tensor_reduce(out=M10[:, 3:4], in_=col0, axis=Ax.XY, op=Alu.add)
                    col1 = XLv[0:64, 259:259 + WP * H].rearrange("p (r w) -> p r w", w=WP)[:, :, 0:1]
                    nc.vector.tensor_reduce(out=M10[:, 4:5], in_=col1, axis=Ax.XY, op=Alu.add)
                else:
                    nc.vector.memset(M10[:, 3:5], 0.0)
                if with_corners:
                    # corners: {132, 259} and {16642, 16769} via stride-127 views
                    nc.vector.tensor_copy(
                        out=M10[:, 5:7].rearrange("p (a b) -> p a b", b=1),
                        in_=XLv[0:64, 132:132 + 254].rearrange("p (a b) -> p a b", b=127)[:, :, 0:1])
                    nc.vector.tensor_copy(
                        out=M10[:, 7:9].rearrange("p (a b) -> p a b", b=1),
                        in_=XLv[0:64, 16642:16642 + 254].rearrange("p (a b) -> p a b", b=127)[:, :, 0:1])
                else:
                    nc.vector.memset(M10[:, 5:9], 0.0)

            if skip_attn or skip_logits_mm:
                nc.vector.memset(attbc, 0.25)
            else:
                # --- logits: 10 accumulating (64->1) matmuls; bias folded into basis 9 ---
                for j in range(10):
                    nc.tensor.matmul(psum_l, M10[:, j:j + 1], cw2_sb[:, j, :],
                                     start=(j == 0), stop=(j == 9))
                nc.vector.tensor_copy(out=att_sb[0:1, :], in_=psum_l)

                # --- softmax (on partition 0) + fold 1/sum into broadcast matmul ---
                nc.vector.tensor_reduce(out=mx, in_=att_sb[0:1, :], axis=Ax.X, op=Alu.max)
                nc.vector.tensor_scalar_mul(out=mxn, in0=mx, scalar1=-1.0)
                nc.scalar.activation(out=att_sb[0:1, :], in_=att_sb[0:1, :], func=Act.Exp,
                                     bias=mxn[0:1, 0:1], scale=1.0)
                nc.vector.tensor_reduce(out=rs, in_=att_sb[0:1, :], axis=Ax.X, op=Alu.add)
                nc.vector.reciprocal(out=rs, in_=rs)
                nc.vector.tensor_scalar_mul(out=onesrow[0:1, :], in0=ones1, scalar1=rs[0:1, 0:1])
                nc.tensor.matmul(psum_b, onesrow, att_sb, start=True, stop=True)
                nc.vector.tensor_copy(out=attbc, in_=psum_b)

            # --- weight mixing: mw = conv_bank + sum_k att_k * bank_k ---
            nc.vector.scalar_tensor_tensor(
                out=mw[:, :, :], in0=wb_sb[:, 0, :].rearrange("p (m c) -> p m c", m=6),
                scalar=attbc[:, 0:1],
                in1=wb_sb[:, 4, :].rearrange("p (m c) -> p m c", m=6),
                op0=Alu.mult, op1=Alu.add)
            for k in range(1, K):
                nc.vector.scalar_tensor_tensor(
                    out=mw[:, :, :], in0=wb_sb[:, k, :].rearrange("p (m c) -> p m c", m=6),
                    scalar=attbc[:, k:k + 1], in1=mw[:, :, :],
                    op0=Alu.mult, op1=Alu.add)
            if conv_dt == "bf16":
                nc.vector.tensor_copy(out=mwb, in_=mw)
                lhs_src = mwb
            else:
                lhs_src = mw

            # --- main conv: 43 PSUM tiles x 6 accumulating matmuls ---
            r0s = [] if skip_conv else list(range(1, H + 1, ROWS_PER_TILE))
            if skip_conv:
                zst = STG.tile([64, H * 2], f32)
                nc.vector.tensor_copy(out=zst[:, 0:K], in_=attbc[0:64, :])
                nc.vector.memset(zst[:, K:H * 2], 0.0)
                for rr in range(0, H, 2):
                    nc.scalar.dma_start(out=outT[:, rr:rr + 2, :],
                                        in_=zst[:, :].rearrange("p (r w) -> p r w", w=W))
            for ti, r0 in enumerate(r0s):
                nrows = min(ROWS_PER_TILE, H + 1 - r0)
                F = WP * nrows
                pt = PS.tile([64, WP * ROWS_PER_TILE], f32, tag="cps", name=f"cps{ti}")
                pt = pt[:, :F]
                for m in range(6):
                    lhsT = lhs_src[:, m, :]
                    rhs = XL[:, WP * r0 + MM_OFFS[m] + 1: WP * r0 + MM_OFFS[m] + 1 + F]
                    nc.tensor.matmul(pt, lhsT, rhs, start=(m == 0), stop=(m == 5))
                st = STG.tile([64, WP * ROWS_PER_TILE], f32, tag="stg", name=f"stg{ti}")
                if ti % 2 == 0:
                    nc.scalar.add(out=st[:, :F], in_=pt, add=convb_sb[:, 0:1])
                else:
                    nc.vector.tensor_scalar_add(out=st[:, :F], in0=pt, scalar1=convb_sb[:, 0:1])
                src = st[:, :F].rearrange("p (r w) -> p r w", w=WP)[:, :, 1:1 + W]
                nc.scalar.dma_start(out=outT[:, r0 - 1:r0 - 1 + nrows, :], in_=src)

    nc.compile()
    return nc


def _get_nc():
    if CONV_DT not in _NC_CACHE:
        _NC_CACHE[CONV_DT] = _build_nc(CONV_DT)
    return _NC_CACHE[CONV_DT]


def _prep_inputs(x, weight, conv_w, conv_b, net0_w, net0_b, net1_w, net1_b,
                 net2_w, net2_b):
    cw2 = _make_cw2(np.asarray(net0_w, np.float32), np.asarray(net0_b, np.float32),
                    np.asarray(net1_w, np.float32), np.asarray(net1_b, np.float32),
                    np.asarray(net2_w, np.float32), np.asarray(net2_b, np.float32))
    banks = np.stack([_make_bank(np.asarray(weight, np.float32)[k]) for k in range(K)]
                     + [_make_bank(np.asarray(conv_w, np.float32))])  # (5,128,6,64)
    banks = np.ascontiguousarray(banks.reshape(5, 128, 6 * 64).transpose(1, 0, 2))
    convb = np.ascontiguousarray(np.asarray(conv_b, np.float32).reshape(C, 1))
    x = np.asarray(x, np.float32)
    if CONV_DT == "bf16":
        import ml_dtypes
        xs = x.astype(ml_dtypes.bfloat16)
    else:
        xs = x
    in_maps = []
    for n in range(N):
        in_maps.append({
            "xin": np.ascontiguousarray(xs[n].reshape(C, H * W)),
            "wbanks": banks,
            "cw2": cw2,
            "convb": convb,
        })
    return in_maps


def _run(inputs, trace=False, **kw):
    from concourse.bass_utils import run_bass_kernel_spmd
    nc = _get_nc()
    in_maps = _prep_inputs(**inputs)
    return run_bass_kernel_spmd(nc, in_maps, core_ids=list(range(N)), trace=trace, **kw)


def kernel(**inputs):
    res = _run(inputs)
    out = np.stack([res.results[n]["out"] for n in range(N)]).astype(np.float32)
    return out


# revision 20
# speedup vs baseline: 1.2510x; 1.2510x over previous
"""CondConv2d on 8 Trainium2 NeuronCores — data-parallel over batch N=8.

Per-core (one sample):
  - attention branch collapsed to 10 "basis" sums of x (total / edge rows /
    edge cols / corners / const) contracted with host-precomputed coefficients
    -> logits -> softmax -> per-sample mixed 3x3 weight, with the static
    residual conv fused in (mw = sum_k att_k W_k + conv_w; bias added at
    PSUM eviction).
  - 3x3 conv as 6 accumulating PE matmuls per PSUM tile over a 130-wide
    zero-padded layout; contraction 128 = 64 channels (lower partitions) +
    64 channels of a row-shifted copy (upper partitions), pairing taps
    (-1,w)+(0,w) per matmul.
"""
import os
import numpy as np

N, C, H, W = 8, 64, 128, 128
K = 4
WP = W + 2                 # padded row width (130)
NELEM = WP * WP + 2        # per-partition x buffer length (16902); xpadflat at elem offset 1
ROWS_PER_TILE = 3          # output rows per PSUM tile (free dim 390 <= 512)
NCHUNKS = 4                # x load/reduce chunks (128 rows / NCHUNKS each)

CONV_DT = os.environ.get("KCONV_DT", "fp32r")   # "fp32" | "fp32r" | "bf16"

MM_TAPS = [((-1, -1), (0, -1)), ((-1, 0), (0, 0)), ((-1, 1), (0, 1)),
           ((1, -1), None), ((1, 0), None), ((1, 1), None)]
MM_OFFS = [130 * L[0] + L[1] for L, _ in MM_TAPS]


# ----------------------------------------------------------------------------
# host-side prep
# ----------------------------------------------------------------------------
def _make_cw2(net0_w, net0_b, net1_w, net1_b, net2_w, net2_b):
    """CW2[c, b, k]: logits[k] = sum_{c,b} CW2[c,b,k] * basis[c,b].
    basis: 0=total, 1=row0, 2=row127, 3=col0, 4=col127,
           5..8=corners (00,0W,H0,HW), 9=const 1."""
    cw = np.zeros((C, 10, K), np.float64)
    scale = 1.0 / (C * H * W)
    for w_net, pads in ((net0_w, (0, 0, 0)), (net1_w, (1, 1, 1)), (net2_w, (2, 1, 1))):
        Kk, _, kd, kh, kw = w_net.shape
        pd, ph, pw = pads
        for i in range(kd):
            clo, chi = max(0, i - pd), min(C - 1, C - 1 + i - pd)
            cmask = np.zeros(C)
            cmask[clo:chi + 1] = 1.0
            for j in range(kh):
                hlo, hhi = max(0, j - ph), min(H - 1, H - 1 + j - ph)
                dropA = 0 if hlo == 1 else (127 if hhi == H - 2 else None)
                for l in range(kw):
                    wlo, whi = max(0, l - pw), min(W - 1, W - 1 + l - pw)
                    dropB = 0 if wlo == 1 else (127 if whi == W - 2 else None)
                    v = np.zeros(10)
                    v[0] = 1.0
                    if dropA == 0: v[1] = -1.0
                    if dropA == 127: v[2] = -1.0
                    if dropB == 0: v[3] = -1.0
                    if dropB == 127: v[4] = -1.0
                    if dropA is not None and dropB is not None:
                        v[{(0, 0): 5, (0, 127): 6, (127, 0): 7, (127, 127): 8}[(dropA, dropB)]] = 1.0
                    for k in range(Kk):
                        cw[:, :, k] += w_net[k, 0, i, j, l] * scale * np.outer(cmask, v)
    btot = (net0_b + net1_b + net2_b).astype(np.float64)
    cw[:, 9, :] += btot[None, :] / C
    return np.ascontiguousarray(cw.astype(np.float32))


def _make_bank(Wt):
    """Wt (co, ci, 3, 3) -> (128, 6, 64): [p=ci(lo)/64+ci(hi), mm, co]."""
    bank = np.zeros((128, 6, 64), np.float32)
    for m, (L, Hh) in enumerate(MM_TAPS):
        bank[:64, m, :] = Wt[:, :, 1 + L[0], 1 + L[1]].T
        if Hh is not None:
            bank[64:, m, :] = Wt[:, :, 1 + Hh[0], 1 + Hh[1]].T
    return bank


# ----------------------------------------------------------------------------
# device program
# ----------------------------------------------------------------------------
_NC_CACHE = {}


def _build_nc(conv_dt, skip_attn=False, skip_conv=False, skip_logits_mm=False,
              spans_mode="mixed", with_edges=True, with_cols=True, with_corners=True):
    import concourse.bacc as bacc
    import concourse.tile as tile
    from concourse import mybir

    f32 = mybir.dt.float32
    if conv_dt == "bf16":
        DT = mybir.dt.bfloat16
    elif conv_dt == "fp32r":
        DT = mybir.dt.float32r
    else:
        DT = f32
    Alu = mybir.AluOpType
    Ax = mybir.AxisListType
    Act = mybir.ActivationFunctionType

    nc = bacc.Bacc("TRN2", target_bir_lowering=False, debug=False,
                   enable_asserts=False, num_devices=N)
    xin = nc.dram_tensor("xin", [C, H * W], DT, kind="ExternalInput")
    wbk = nc.dram_tensor("wbanks", [128, 5, 6 * 64], f32, kind="ExternalInput")
    cw2 = nc.dram_tensor("cw2", [C, 10, K], f32, kind="ExternalInput")
    cb = nc.dram_tensor("convb", [C, 1], f32, kind="ExternalInput")
    outT = nc.dram_tensor("out", [C, H, W], f32, kind="ExternalOutput")

    rows_per_chunk = H // NCHUNKS                      # 32
    span_elems = WP * rows_per_chunk                   # 4160

    with tile.TileContext(nc) as tc:
        with tc.tile_pool(name="singles", bufs=1) as S, \
             tc.tile_pool(name="stage", bufs=4) as STG, \
             tc.tile_pool(name="cpsum", bufs=4, space="PSUM") as PS, \
             tc.tile_pool(name="spsum", bufs=1, space="PSUM") as PS1:

            XL = S.tile([128, NELEM], DT)
            wb_sb = S.tile([128, 5, 6 * 64], f32)
            cw2_sb = S.tile([C, 10, K], f32)
            convb_sb = S.tile([C, 1], f32)
            onesrow = S.tile([128, 128], f32)
            att_sb = S.tile([128, K], f32)
            attbc = S.tile([128, K], f32)
            M10 = S.tile([C, 10], f32)
            PART = S.tile([C, NCHUNKS], f32)
            mw = None  # allocated below once MWDT is known
            mwb = S.tile([128, 6, 64], DT, name="mwb") if conv_dt == "bf16" else None
            fold = S.tile([C, 2600], f32)
            actout = S.tile([C, 3300], f32)
            rs128 = S.tile([128, 1], f32)

            wpsum = PS1.tile([128, 128], f32)
            psum_b = PS1.tile([128, K], f32)

            XLv = XL.bitcast(f32) if conv_dt == "fp32r" else XL
            MWDT = mybir.dt.float32r if conv_dt == "fp32r" else f32
            mw = S.tile([128, 6, 64], MWDT, name="mw")
            # --- constants / border zeroing (DVE, all tiny) ---
            nc.vector.memset(onesrow, 0.0)
            nc.vector.memset(M10[:, 9:10], 1.0)
            # lower borders: head, row gaps (2 elems each), tail
            nc.vector.memset(XLv[0:64, 0:132], 0.0)
            nc.vector.memset(
                XLv[0:64, 260:260 + WP * 127].rearrange("p (r w) -> p r w", w=WP)[:, :, 0:2], 0.0)
            nc.vector.memset(XLv[0:64, 16770:NELEM], 0.0)
            # upper borders
            nc.vector.memset(XLv[64:128, 0:2], 0.0)
            nc.vector.memset(
                XLv[64:128, WP:WP + WP * 127].rearrange("p (r w) -> p r w", w=WP)[:, :, 0:2], 0.0)
            nc.vector.memset(XLv[64:128, 16640:NELEM], 0.0)

            # --- small input DMAs ---
            nc.gpsimd.dma_start(out=wb_sb, in_=wbk[:, :, :])
            nc.gpsimd.dma_start(out=cw2_sb, in_=cw2[:, :, :])
            nc.gpsimd.dma_start(out=convb_sb, in_=cb[:, :])

            # --- x load (lower: offset 132; upper row-shifted copy: offset 2) ---
            xsrc = xin[:, :].rearrange("p (r w) -> p r w", w=W)
            for c in range(NCHUNKS):
                r0 = c * rows_per_chunk
                dst_lo = XL[0:64, 132 + span_elems * c: 132 + span_elems * (c + 1)] \
                    .rearrange("p (r w) -> p r w", w=WP)[:, :, 0:W]
                nc.sync.dma_start(out=dst_lo, in_=xsrc[:, r0:r0 + rows_per_chunk, :])
                dst_hi = XL[64:128, 2 + span_elems * c: 2 + span_elems * (c + 1)] \
                    .rearrange("p (r w) -> p r w", w=WP)[:, :, 0:W]
                nc.scalar.dma_start(out=dst_hi, in_=xsrc[:, r0:r0 + rows_per_chunk, :])

            # --- PE warm-up (results discarded; onesrow is all-zero here) ---
            for i in range(4):
                nc.tensor.matmul(wpsum, onesrow, onesrow, start=True, stop=True)
            for c in range(NCHUNKS):
                a = 132 + span_elems * c
                for i in range(2):
                    # both operands from XL so the warm-up rides the chunk-DMA deps
                    nc.tensor.matmul(wpsum, XL[:, a:a + 128], XL[:, a + 128 * i:a + 128 * i + 128],
                                     start=True, stop=True)

            # --- attention basis sums ---
            # DVE: scalar_tensor_tensor fold (reads 2 streams/cycle) with accum_out;
            # ACT: activation-Identity with accum_out.  (tensor_tensor_reduce is
            # broken on this runtime — do not use.)
            spans = [(0, 5200, "dve"), (5200, 5200, "dve"),
                     (10400, 3250, "act"), (13650, NELEM - 13650, "act")]
            if skip_attn:
                spans = []
            for idx, (a, ln, eng) in enumerate(spans):
                if spans_mode == "basic":
                    eng = "basic"
                elif spans_mode == "act":
                    eng = "act"
                if eng == "dve":
                    h = ln // 2
                    nc.vector.scalar_tensor_tensor(
                        out=fold[:, :h], in0=XLv[0:64, a:a + h], scalar=1.0,
                        in1=XLv[0:64, a + h:a + ln], op0=Alu.mult, op1=Alu.add,
                        accum_out=PART[:, idx:idx + 1])
                elif eng == "act":
                    nc.scalar.activation(
                        out=actout[:, :ln], in_=XLv[0:64, a:a + ln], func=Act.Identity,
                        bias=0.0, scale=1.0, accum_out=PART[:, idx:idx + 1])
                else:
                    nc.vector.tensor_reduce(out=PART[:, idx:idx + 1], in_=XLv[0:64, a:a + ln],
                                            axis=Ax.X, op=Alu.add)
            if not skip_attn:
                nc.vector.tensor_reduce(out=M10[:, 0:1], in_=PART, axis=Ax.X, op=Alu.add)
                if with_edges:
                    nc.vector.tensor_reduce(out=M10[:, 1:2], in_=XLv[0:64, 132:132 + W], axis=Ax.X, op=Alu.add)
                    nc.vector.tensor_reduce(out=M10[:, 2:3], in_=XLv[0:64, 16642:16642 + W], axis=Ax.X, op=Alu.add)
                else:
                    nc.vector.memset(M10[:, 1:3], 0.0)
                if with_cols:
                    col0 = XLv[0:64, 132:132 + WP * H].rearrange("p (r w) -> p r w", w=WP)[:, :, 0:1]
                    nc.vector.tensor_reduce(out=M10[:, 3:4], in_=col0, axis=Ax.XY, op=Alu.add)
                    col1 = XLv[0:64, 259:259 + WP * H].rearrange("p (r w) -> p r w", w=WP)[:, :, 0:1]
                    nc.vector.tensor_reduce(out=M10[:, 4:5], in_=col1, axis=Ax.XY, op=Alu.add)
                else:
                    nc.vector.memset(M10[:, 3:5], 0.0)
                if with_corners:
                    # corners: {132, 259} and {16642, 16769} via stride-127 views
                    nc.vector.tensor_copy(
                        out=M10[:, 5:7].rearrange("p (a b) -> p a b", b=1),
                        in_=XLv[0:64, 132:132 + 254].rearrange("p (a b) -> p a b", b=127)[:, :, 0:1])
                    nc.vector.tensor_copy(
                        out=M10[:, 7:9].rearrange("p (a b) -> p a b", b=1),
                        in_=XLv[0:64, 16642:16642 + 254].rearrange("p (a b) -> p a b", b=127)[:, :, 0:1])
                else:
                    nc.vector.memset(M10[:, 5:9], 0.0)

            if skip_attn or skip_logits_mm:
                nc.vector.memset(attbc, 0.25)
            else:
                # --- logits, replicated on all 128 partitions via stride-0 lhsT ---
                for j in range(10):
                    nc.tensor.matmul(psum_b, M10[:, j:j + 1].to_broadcast((C, 128)),
                                     cw2_sb[:, j, :], start=(j == 0), stop=(j == 9))
                # softmax per partition (all partitions identical); logits are tiny,
                # so skip the max-subtraction
                nc.scalar.activation(out=att_sb, in_=psum_b, func=Act.Exp)
                nc.vector.tensor_reduce(out=rs128, in_=att_sb, axis=Ax.X, op=Alu.add)
                nc.vector.reciprocal(out=rs128, in_=rs128)
                nc.vector.tensor_scalar_mul(out=attbc, in0=att_sb, scalar1=rs128)

            # --- weight mixing: mw = conv_bank + sum_k att_k * bank_k ---
            nc.vector.scalar_tensor_tensor(
                out=mw[:, :, :], in0=wb_sb[:, 0, :].rearrange("p (m c) -> p m c", m=6),
                scalar=attbc[:, 0:1],
                in1=wb_sb[:, 4, :].rearrange("p (m c) -> p m c", m=6),
                op0=Alu.mult, op1=Alu.add)
            for k in range(1, K):
                nc.vector.scalar_tensor_tensor(
                    out=mw[:, :, :], in0=wb_sb[:, k, :].rearrange("p (m c) -> p m c", m=6),
                    scalar=attbc[:, k:k + 1], in1=mw[:, :, :],
                    op0=Alu.mult, op1=Alu.add)
            if conv_dt == "bf16":
                nc.vector.tensor_copy(out=mwb, in_=mw)
                lhs_src = mwb
            else:
                lhs_src = mw

            # --- main conv: 43 PSUM tiles x 6 accumulating matmuls ---
            r0s = [] if skip_conv else list(range(1, H + 1, ROWS_PER_TILE))
            if skip_conv:
                zst = STG.tile([64, H * 2], f32)
                nc.vector.tensor_copy(out=zst[:, 0:K], in_=attbc[0:64, :])
                nc.vector.memset(zst[:, K:H * 2], 0.0)
                for rr in range(0, H, 2):
                    nc.scalar.dma_start(out=outT[:, rr:rr + 2, :],
                                        in_=zst[:, :].rearrange("p (r w) -> p r w", w=W))
            for ti, r0 in enumerate(r0s):
                nrows = min(ROWS_PER_TILE, H + 1 - r0)
                F = WP * nrows
                pt = PS.tile([64, WP * ROWS_PER_TILE], f32, tag="cps", name=f"cps{ti}")
                pt = pt[:, :F]
                for m in range(6):
                    lhsT = lhs_src[:, m, :]
                    rhs = XL[:, WP * r0 + MM_OFFS[m] + 1: WP * r0 + MM_OFFS[m] + 1 + F]
                    nc.tensor.matmul(pt, lhsT, rhs, start=(m == 0), stop=(m == 5))
                st = STG.tile([64, WP * ROWS_PER_TILE], f32, tag="stg", name=f"stg{ti}")
                if ti % 2 == 0:
                    nc.scalar.add(out=st[:, :F], in_=pt, add=convb_sb[:, 0:1])
                else:
                    nc.vector.tensor_scalar_add(out=st[:, :F], in0=pt, scalar1=convb_sb[:, 0:1])
                src = st[:, :F].rearrange("p (r w) -> p r w", w=WP)[:, :, 1:1 + W]
                eng = nc.sync if ti % 2 == 0 else nc.scalar
                eng.dma_start(out=outT[:, r0 - 1:r0 - 1 + nrows, :], in_=src)

    nc.compile()
    return nc


def _get_nc():
    if CONV_DT not in _NC_CACHE:
        _NC_CACHE[CONV_DT] = _build_nc(CONV_DT)
    return _NC_CACHE[CONV_DT]


def _prep_inputs(x, weight, conv_w, conv_b, net0_w, net0_b, net1_w, net1_b,
                 net2_w, net2_b):
    cw2 = _make_cw2(np.asarray(net0_w, np.float32), np.asarray(net0_b, np.float32),
                    np.asarray(net1_w, np.float32), np.asarray(net1_b, np.float32),
                    np.asarray(net2_w, np.float32), np.asarray(net2_b, np.float32))
    banks = np.stack([_make_bank(np.asarray(weight, np.float32)[k]) for k in range(K)]
                     + [_make_bank(np.asarray(conv_w, np.float32))])  # (5,128,6,64)
    banks = np.ascontiguousarray(banks.reshape(5, 128, 6 * 64).transpose(1, 0, 2))
    convb = np.ascontiguousarray(np.asarray(conv_b, np.float32).reshape(C, 1))
    x = np.asarray(x, np.float32)
    if CONV_DT == "bf16":
        import ml_dtypes
        xs = x.astype(ml_dtypes.bfloat16)
    else:
        xs = x
    in_maps = []
    for n in range(N):
        in_maps.append({
            "xin": np.ascontiguousarray(xs[n].reshape(C, H * W)),
            "wbanks": banks,
            "cw2": cw2,
            "convb": convb,
        })
    return in_maps


def _run(inputs, trace=False, **kw):
    from concourse.bass_utils import run_bass_kernel_spmd
    nc = _get_nc()
    in_maps = _prep_inputs(**inputs)
    return run_bass_kernel_spmd(nc, in_maps, core_ids=list(range(N)), trace=trace, **kw)


def kernel(**inputs):
    res = _run(inputs)
    out = np.stack([res.results[n]["out"] for n in range(N)]).astype(np.float32)
    return out


# revision 21
# speedup vs baseline: 1.3093x; 1.0466x over previous
"""CondConv2d on 8 Trainium2 NeuronCores — data-parallel over batch N=8.

Per-core (one sample):
  - attention branch collapsed to 10 "basis" sums of x (total / edge rows /
    edge cols / corners / const) contracted with host-precomputed coefficients
    -> logits -> softmax -> per-sample mixed 3x3 weight, with the static
    residual conv fused in (mw = sum_k att_k W_k + conv_w; bias added at
    PSUM eviction).
  - 3x3 conv as 6 accumulating PE matmuls per PSUM tile over a 130-wide
    zero-padded layout; contraction 128 = 64 channels (lower partitions) +
    64 channels of a row-shifted copy (upper partitions), pairing taps
    (-1,w)+(0,w) per matmul.
"""
import os
import numpy as np

N, C, H, W = 8, 64, 128, 128
K = 4
WP = W + 2                 # padded row width (130)
NELEM = WP * WP + 2        # per-partition x buffer length (16902); xpadflat at elem offset 1
ROWS_PER_TILE = 3          # output rows per PSUM tile (free dim 390 <= 512)
NCHUNKS = 4                # x load/reduce chunks (128 rows / NCHUNKS each)

CONV_DT = os.environ.get("KCONV_DT", "fp32r")   # "fp32" | "fp32r" | "bf16"

MM_TAPS = [((-1, -1), (0, -1)), ((-1, 0), (0, 0)), ((-1, 1), (0, 1)),
           ((1, -1), None), ((1, 0), None), ((1, 1), None)]
MM_OFFS = [130 * L[0] + L[1] for L, _ in MM_TAPS]


# ----------------------------------------------------------------------------
# host-side prep
# ----------------------------------------------------------------------------
def _make_cw2(net0_w, net0_b, net1_w, net1_b, net2_w, net2_b):
    """CW2[c, b, k]: logits[k] = sum_{c,b} CW2[c,b,k] * basis[c,b].
    basis: 0=total, 1=row0, 2=row127, 3=col0, 4=col127,
           5..8=corners (00,0W,H0,HW), 9=const 1."""
    cw = np.zeros((C, 10, K), np.float64)
    scale = 1.0 / (C * H * W)
    for w_net, pads in ((net0_w, (0, 0, 0)), (net1_w, (1, 1, 1)), (net2_w, (2, 1, 1))):
        Kk, _, kd, kh, kw = w_net.shape
        pd, ph, pw = pads
        for i in range(kd):
            clo, chi = max(0, i - pd), min(C - 1, C - 1 + i - pd)
            cmask = np.zeros(C)
            cmask[clo:chi + 1] = 1.0
            for j in range(kh):
                hlo, hhi = max(0, j - ph), min(H - 1, H - 1 + j - ph)
                dropA = 0 if hlo == 1 else (127 if hhi == H - 2 else None)
                for l in range(kw):
                    wlo, whi = max(0, l - pw), min(W - 1, W - 1 + l - pw)
                    dropB = 0 if wlo == 1 else (127 if whi == W - 2 else None)
                    v = np.zeros(10)
                    v[0] = 1.0
                    if dropA == 0: v[1] = -1.0
                    if dropA == 127: v[2] = -1.0
                    if dropB == 0: v[3] = -1.0
                    if dropB == 127: v[4] = -1.0
                    if dropA is not None and dropB is not None:
                        v[{(0, 0): 5, (0, 127): 6, (127, 0): 7, (127, 127): 8}[(dropA, dropB)]] = 1.0
                    for k in range(Kk):
                        cw[:, :, k] += w_net[k, 0, i, j, l] * scale * np.outer(cmask, v)
    btot = (net0_b + net1_b + net2_b).astype(np.float64)
    cw[:, 9, :] += btot[None, :] / C
    return np.ascontiguousarray(cw.astype(np.float32))


def _make_bank(Wt):
    """Wt (co, ci, 3, 3) -> (128, 6, 64): [p=ci(lo)/64+ci(hi), mm, co]."""
    bank = np.zeros((128, 6, 64), np.float32)
    for m, (L, Hh) in enumerate(MM_TAPS):
        bank[:64, m, :] = Wt[:, :, 1 + L[0], 1 + L[1]].T
        if Hh is not None:
            bank[64:, m, :] = Wt[:, :, 1 + Hh[0], 1 + Hh[1]].T
    return bank


# ----------------------------------------------------------------------------
# device program
# ----------------------------------------------------------------------------
_NC_CACHE = {}


def _build_nc(conv_dt, skip_attn=False, skip_conv=False, skip_logits_mm=False,
              spans_mode="mixed", with_edges=True, with_cols=True, with_corners=True):
    import concourse.bacc as bacc
    import concourse.tile as tile
    from concourse import mybir

    f32 = mybir.dt.float32
    if conv_dt == "bf16":
        DT = mybir.dt.bfloat16
    elif conv_dt == "fp32r":
        DT = mybir.dt.float32r
    else:
        DT = f32
    Alu = mybir.AluOpType
    Ax = mybir.AxisListType
    Act = mybir.ActivationFunctionType

    nc = bacc.Bacc("TRN2", target_bir_lowering=False, debug=False,
                   enable_asserts=False, num_devices=N)
    xin = nc.dram_tensor("xin", [C, H * WP], DT, kind="ExternalInput")
    wbk = nc.dram_tensor("wbanks", [128, 5, 6 * 64], f32, kind="ExternalInput")
    cw2 = nc.dram_tensor("cw2", [C, 10, K], f32, kind="ExternalInput")
    cb = nc.dram_tensor("convb", [C, 1], f32, kind="ExternalInput")
    outT = nc.dram_tensor("out", [C, H, W], f32, kind="ExternalOutput")

    rows_per_chunk = H // NCHUNKS                      # 32
    span_elems = WP * rows_per_chunk                   # 4160

    with tile.TileContext(nc) as tc:
        with tc.tile_pool(name="singles", bufs=1) as S, \
             tc.tile_pool(name="stage", bufs=4) as STG, \
             tc.tile_pool(name="cpsum", bufs=4, space="PSUM") as PS, \
             tc.tile_pool(name="spsum", bufs=1, space="PSUM") as PS1:

            XL = S.tile([128, NELEM], DT)
            wb_sb = S.tile([128, 5, 6 * 64], f32)
            cw2_sb = S.tile([C, 10, K], f32)
            convb_sb = S.tile([C, 1], f32)
            onesrow = S.tile([128, 128], f32)
            att_sb = S.tile([128, K], f32)
            attbc = S.tile([128, K], f32)
            M10 = S.tile([C, 10], f32)
            PART = S.tile([C, NCHUNKS], f32)
            mw = None  # allocated below once MWDT is known
            mwb = S.tile([128, 6, 64], DT, name="mwb") if conv_dt == "bf16" else None
            fold = S.tile([C, 2600], f32)
            actout = S.tile([C, 3300], f32)
            rs128 = S.tile([128, 1], f32)

            wpsum = PS1.tile([128, 128], f32)
            psum_b = PS1.tile([128, K], f32)

            XLv = XL.bitcast(f32) if conv_dt == "fp32r" else XL
            MWDT = mybir.dt.float32r if conv_dt == "fp32r" else f32
            mw = S.tile([128, 6, 64], MWDT, name="mw")
            # --- constants / border zeroing (DVE, all tiny) ---
            nc.vector.memset(onesrow, 0.0)
            nc.vector.memset(M10[:, 9:10], 1.0)
            # borders: host pre-pads the row gaps; only head/tail need zeroing
            nc.vector.memset(XLv[0:64, 0:132], 0.0)
            nc.vector.memset(XLv[0:64, 132 + H * WP:NELEM], 0.0)
            nc.vector.memset(XLv[64:128, 0:2], 0.0)
            nc.vector.memset(XLv[64:128, 2 + H * WP:NELEM], 0.0)

            # --- small input DMAs ---
            nc.scalar.dma_start(out=wb_sb, in_=wbk[:, :, :])
            nc.scalar.dma_start(out=cw2_sb, in_=cw2[:, :, :])
            nc.scalar.dma_start(out=convb_sb, in_=cb[:, :])

            # --- x load: fully contiguous (host pre-padded 130-wide rows) ---
            # lower copy at offset 132 (sync ring), upper row-shifted copy at
            # offset 2 (gpsimd/SWDGE ring) so the two streams overlap
            for c in range(NCHUNKS):
                a = span_elems * c
                nc.sync.dma_start(out=XL[0:64, 132 + a: 132 + a + span_elems],
                                  in_=xin[:, a: a + span_elems])
                nc.gpsimd.dma_start(out=XL[64:128, 2 + a: 2 + a + span_elems],
                                    in_=xin[:, a: a + span_elems])

            # --- PE warm-up (results discarded; onesrow is all-zero here) ---
            for i in range(4):
                nc.tensor.matmul(wpsum, onesrow, onesrow, start=True, stop=True)
            for c in range(NCHUNKS):
                a = 132 + span_elems * c
                for i in range(2):
                    # both operands from XL so the warm-up rides the chunk-DMA deps
                    nc.tensor.matmul(wpsum, XL[:, a:a + 128], XL[:, a + 128 * i:a + 128 * i + 128],
                                     start=True, stop=True)

            # --- attention basis sums ---
            # DVE: scalar_tensor_tensor fold (reads 2 streams/cycle) with accum_out;
            # ACT: activation-Identity with accum_out.  (tensor_tensor_reduce is
            # broken on this runtime — do not use.)
            spans = [(0, 5200, "dve"), (5200, 5200, "dve"),
                     (10400, 3250, "act"), (13650, NELEM - 13650, "act")]
            if skip_attn:
                spans = []
            for idx, (a, ln, eng) in enumerate(spans):
                if spans_mode == "basic":
                    eng = "basic"
                elif spans_mode == "act":
                    eng = "act"
                if eng == "dve":
                    h = ln // 2
                    nc.vector.scalar_tensor_tensor(
                        out=fold[:, :h], in0=XLv[0:64, a:a + h], scalar=1.0,
                        in1=XLv[0:64, a + h:a + ln], op0=Alu.mult, op1=Alu.add,
                        accum_out=PART[:, idx:idx + 1])
                elif eng == "act":
                    nc.scalar.activation(
                        out=actout[:, :ln], in_=XLv[0:64, a:a + ln], func=Act.Identity,
                        bias=0.0, scale=1.0, accum_out=PART[:, idx:idx + 1])
                else:
                    nc.vector.tensor_reduce(out=PART[:, idx:idx + 1], in_=XLv[0:64, a:a + ln],
                                            axis=Ax.X, op=Alu.add)
            if not skip_attn:
                nc.vector.tensor_reduce(out=M10[:, 0:1], in_=PART, axis=Ax.X, op=Alu.add)
                if with_edges:
                    nc.vector.tensor_reduce(out=M10[:, 1:2], in_=XLv[0:64, 132:132 + W], axis=Ax.X, op=Alu.add)
                    nc.vector.tensor_reduce(out=M10[:, 2:3], in_=XLv[0:64, 16642:16642 + W], axis=Ax.X, op=Alu.add)
                else:
                    nc.vector.memset(M10[:, 1:3], 0.0)
                if with_cols:
                    col0 = XLv[0:64, 132:132 + WP * H].rearrange("p (r w) -> p r w", w=WP)[:, :, 0:1]
                    nc.vector.tensor_reduce(out=M10[:, 3:4], in_=col0, axis=Ax.XY, op=Alu.add)
                    col1 = XLv[0:64, 259:259 + WP * H].rearrange("p (r w) -> p r w", w=WP)[:, :, 0:1]
                    nc.vector.tensor_reduce(out=M10[:, 4:5], in_=col1, axis=Ax.XY, op=Alu.add)
                else:
                    nc.vector.memset(M10[:, 3:5], 0.0)
                if with_corners:
                    # corners: {132, 259} and {16642, 16769} via stride-127 views
                    nc.vector.tensor_copy(
                        out=M10[:, 5:7].rearrange("p (a b) -> p a b", b=1),
                        in_=XLv[0:64, 132:132 + 254].rearrange("p (a b) -> p a b", b=127)[:, :, 0:1])
                    nc.vector.tensor_copy(
                        out=M10[:, 7:9].rearrange("p (a b) -> p a b", b=1),
                        in_=XLv[0:64, 16642:16642 + 254].rearrange("p (a b) -> p a b", b=127)[:, :, 0:1])
                else:
                    nc.vector.memset(M10[:, 5:9], 0.0)

            if skip_attn or skip_logits_mm:
                nc.vector.memset(attbc, 0.25)
            else:
                # --- logits, replicated on all 128 partitions via stride-0 lhsT ---
                for j in range(10):
                    nc.tensor.matmul(psum_b, M10[:, j:j + 1].to_broadcast((C, 128)),
                                     cw2_sb[:, j, :], start=(j == 0), stop=(j == 9))
                # softmax per partition (all partitions identical); logits are tiny,
                # so skip the max-subtraction
                nc.scalar.activation(out=att_sb, in_=psum_b, func=Act.Exp)
                nc.vector.tensor_reduce(out=rs128, in_=att_sb, axis=Ax.X, op=Alu.add)
                nc.vector.reciprocal(out=rs128, in_=rs128)
                nc.vector.tensor_scalar_mul(out=attbc, in0=att_sb, scalar1=rs128)

            # --- weight mixing: mw = conv_bank + sum_k att_k * bank_k ---
            nc.vector.scalar_tensor_tensor(
                out=mw[:, :, :], in0=wb_sb[:, 0, :].rearrange("p (m c) -> p m c", m=6),
                scalar=attbc[:, 0:1],
                in1=wb_sb[:, 4, :].rearrange("p (m c) -> p m c", m=6),
                op0=Alu.mult, op1=Alu.add)
            for k in range(1, K):
                nc.vector.scalar_tensor_tensor(
                    out=mw[:, :, :], in0=wb_sb[:, k, :].rearrange("p (m c) -> p m c", m=6),
                    scalar=attbc[:, k:k + 1], in1=mw[:, :, :],
                    op0=Alu.mult, op1=Alu.add)
            if conv_dt == "bf16":
                nc.vector.tensor_copy(out=mwb, in_=mw)
                lhs_src = mwb
            else:
                lhs_src = mw

            # --- main conv: 43 PSUM tiles x 6 accumulating matmuls ---
            r0s = [] if skip_conv else list(range(1, H + 1, ROWS_PER_TILE))
            if skip_conv:
                zst = STG.tile([64, H * 2], f32)
                nc.vector.tensor_copy(out=zst[:, 0:K], in_=attbc[0:64, :])
                nc.vector.memset(zst[:, K:H * 2], 0.0)
                for rr in range(0, H, 2):
                    nc.scalar.dma_start(out=outT[:, rr:rr + 2, :],
                                        in_=zst[:, :].rearrange("p (r w) -> p r w", w=W))
            for ti, r0 in enumerate(r0s):
                nrows = min(ROWS_PER_TILE, H + 1 - r0)
                F = WP * nrows
                pt = PS.tile([64, WP * ROWS_PER_TILE], f32, tag="cps", name=f"cps{ti}")
                pt = pt[:, :F]
                for m in range(6):
                    lhsT = lhs_src[:, m, :]
                    rhs = XL[:, WP * r0 + MM_OFFS[m] + 1: WP * r0 + MM_OFFS[m] + 1 + F]
                    nc.tensor.matmul(pt, lhsT, rhs, start=(m == 0), stop=(m == 5))
                st = STG.tile([64, WP * ROWS_PER_TILE], f32, tag="stg", name=f"stg{ti}")
                if ti % 2 == 0:
                    nc.scalar.add(out=st[:, :F], in_=pt, add=convb_sb[:, 0:1])
                else:
                    nc.vector.tensor_scalar_add(out=st[:, :F], in0=pt, scalar1=convb_sb[:, 0:1])
                src = st[:, :F].rearrange("p (r w) -> p r w", w=WP)[:, :, 1:1 + W]
                eng = nc.sync if ti % 2 == 0 else nc.scalar
                eng.dma_start(out=outT[:, r0 - 1:r0 - 1 + nrows, :], in_=src)

    nc.compile()
    return nc


def _get_nc():
    if CONV_DT not in _NC_CACHE:
        _NC_CACHE[CONV_DT] = _build_nc(CONV_DT)
    return _NC_CACHE[CONV_DT]


def _prep_inputs(x, weight, conv_w, conv_b, net0_w, net0_b, net1_w, net1_b,
                 net2_w, net2_b):
    cw2 = _make_cw2(np.asarray(net0_w, np.float32), np.asarray(net0_b, np.float32),
                    np.asarray(net1_w, np.float32), np.asarray(net1_b, np.float32),
                    np.asarray(net2_w, np.float32), np.asarray(net2_b, np.float32))
    banks = np.stack([_make_bank(np.asarray(weight, np.float32)[k]) for k in range(K)]
                     + [_make_bank(np.asarray(conv_w, np.float32))])  # (5,128,6,64)
    banks = np.ascontiguousarray(banks.reshape(5, 128, 6 * 64).transpose(1, 0, 2))
    convb = np.ascontiguousarray(np.asarray(conv_b, np.float32).reshape(C, 1))
    x = np.asarray(x, np.float32)
    xp = np.zeros((N, C, H, WP), np.float32)
    xp[:, :, :, :W] = x
    if CONV_DT == "bf16":
        import ml_dtypes
        xs = xp.astype(ml_dtypes.bfloat16)
    else:
        xs = xp
    in_maps = []
    for n in range(N):
        in_maps.append({
            "xin": np.ascontiguousarray(xs[n].reshape(C, H * WP)),
            "wbanks": banks,
            "cw2": cw2,
            "convb": convb,
        })
    return in_maps


def _run(inputs, trace=False, **kw):
    from concourse.bass_utils import run_bass_kernel_spmd
    nc = _get_nc()
    in_maps = _prep_inputs(**inputs)
    return run_bass_kernel_spmd(nc, in_maps, core_ids=list(range(N)), trace=trace, **kw)


def kernel(**inputs):
    res = _run(inputs)
    out = np.stack([res.results[n]["out"] for n in range(N)]).astype(np.float32)
    return out


# revision 23
# speedup vs baseline: 1.3281x; 1.0144x over previous
"""CondConv2d on 8 Trainium2 NeuronCores — data-parallel over batch N=8.

Per-core (one sample):
  - attention branch collapsed to 10 "basis" sums of x (total / edge rows /
    edge cols / corners / const) contracted with host-precomputed coefficients
    -> logits -> softmax -> per-sample mixed 3x3 weight, with the static
    residual conv fused in (mw = sum_k att_k W_k + conv_w; bias added at
    PSUM eviction).
  - 3x3 conv as 6 accumulating PE matmuls per PSUM tile over a 130-wide
    zero-padded layout; contraction 128 = 64 channels (lower partitions) +
    64 channels of a row-shifted copy (upper partitions), pairing taps
    (-1,w)+(0,w) per matmul.
"""
import os
import numpy as np

N, C, H, W = 8, 64, 128, 128
K = 4
WP = W + 2                 # padded row width (130)
NELEM = WP * WP + 2        # per-partition x buffer length (16902); xpadflat at elem offset 1
ROWS_PER_TILE = 3          # output rows per PSUM tile (free dim 390 <= 512)
NCHUNKS = 4                # x load/reduce chunks (128 rows / NCHUNKS each)

CONV_DT = os.environ.get("KCONV_DT", "fp32r")   # "fp32" | "fp32r" | "bf16"

MM_TAPS = [((-1, -1), (0, -1)), ((-1, 0), (0, 0)), ((-1, 1), (0, 1)),
           ((1, -1), None), ((1, 0), None), ((1, 1), None)]
MM_OFFS = [130 * L[0] + L[1] for L, _ in MM_TAPS]


# ----------------------------------------------------------------------------
# host-side prep
# ----------------------------------------------------------------------------
def _make_cw2(net0_w, net0_b, net1_w, net1_b, net2_w, net2_b):
    """CW2[c, b, k]: logits[k] = sum_{c,b} CW2[c,b,k] * basis[c,b].
    basis: 0=total, 1=row0, 2=row127, 3=col0, 4=col127,
           5..8=corners (00,0W,H0,HW), 9=const 1."""
    cw = np.zeros((C, 10, K), np.float64)
    scale = 1.0 / (C * H * W)
    for w_net, pads in ((net0_w, (0, 0, 0)), (net1_w, (1, 1, 1)), (net2_w, (2, 1, 1))):
        Kk, _, kd, kh, kw = w_net.shape
        pd, ph, pw = pads
        for i in range(kd):
            clo, chi = max(0, i - pd), min(C - 1, C - 1 + i - pd)
            cmask = np.zeros(C)
            cmask[clo:chi + 1] = 1.0
            for j in range(kh):
                hlo, hhi = max(0, j - ph), min(H - 1, H - 1 + j - ph)
                dropA = 0 if hlo == 1 else (127 if hhi == H - 2 else None)
                for l in range(kw):
                    wlo, whi = max(0, l - pw), min(W - 1, W - 1 + l - pw)
                    dropB = 0 if wlo == 1 else (127 if whi == W - 2 else None)
                    v = np.zeros(10)
                    v[0] = 1.0
                    if dropA == 0: v[1] = -1.0
                    if dropA == 127: v[2] = -1.0
                    if dropB == 0: v[3] = -1.0
                    if dropB == 127: v[4] = -1.0
                    if dropA is not None and dropB is not None:
                        v[{(0, 0): 5, (0, 127): 6, (127, 0): 7, (127, 127): 8}[(dropA, dropB)]] = 1.0
                    for k in range(Kk):
                        cw[:, :, k] += w_net[k, 0, i, j, l] * scale * np.outer(cmask, v)
    btot = (net0_b + net1_b + net2_b).astype(np.float64)
    cw[:, 9, :] += btot[None, :] / C
    return np.ascontiguousarray(cw.astype(np.float32))


def _make_bank(Wt):
    """Wt (co, ci, 3, 3) -> (128, 6, 64): [p=ci(lo)/64+ci(hi), mm, co]."""
    bank = np.zeros((128, 6, 64), np.float32)
    for m, (L, Hh) in enumerate(MM_TAPS):
        bank[:64, m, :] = Wt[:, :, 1 + L[0], 1 + L[1]].T
        if Hh is not None:
            bank[64:, m, :] = Wt[:, :, 1 + Hh[0], 1 + Hh[1]].T
    return bank


# ----------------------------------------------------------------------------
# device program
# ----------------------------------------------------------------------------
_NC_CACHE = {}


def _build_nc(conv_dt, skip_attn=False, skip_conv=False, skip_logits_mm=False,
              spans_mode="mixed", with_edges=True, with_cols=True, with_corners=True):
    import concourse.bacc as bacc
    import concourse.tile as tile
    from concourse import mybir

    f32 = mybir.dt.float32
    if conv_dt == "bf16":
        DT = mybir.dt.bfloat16
    elif conv_dt == "fp32r":
        DT = mybir.dt.float32r
    else:
        DT = f32
    Alu = mybir.AluOpType
    Ax = mybir.AxisListType
    Act = mybir.ActivationFunctionType

    nc = bacc.Bacc("TRN2", target_bir_lowering=False, debug=False,
                   enable_asserts=False, num_devices=N)
    xin = nc.dram_tensor("xin", [C, H * WP], DT, kind="ExternalInput")
    wbk = nc.dram_tensor("wbanks", [128, 5, 6 * 64], f32, kind="ExternalInput")
    cw2 = nc.dram_tensor("cw2", [C, 10, K], f32, kind="ExternalInput")
    cb = nc.dram_tensor("convb", [C, 1], f32, kind="ExternalInput")
    outT = nc.dram_tensor("out", [C, H, W], f32, kind="ExternalOutput")

    rows_per_chunk = H // NCHUNKS                      # 32
    span_elems = WP * rows_per_chunk                   # 4160

    with tile.TileContext(nc) as tc:
        with tc.tile_pool(name="singles", bufs=1) as S, \
             tc.tile_pool(name="stage", bufs=4) as STG, \
             tc.tile_pool(name="cpsum", bufs=4, space="PSUM") as PS, \
             tc.tile_pool(name="spsum", bufs=1, space="PSUM") as PS1:

            XL = S.tile([128, NELEM], DT)
            wb_sb = S.tile([128, 5, 6 * 64], f32)
            cw2_sb = S.tile([C, 10, K], f32)
            convb_sb = S.tile([C, 1], f32)
            onesrow = S.tile([128, 128], f32)
            att_sb = S.tile([128, K], f32)
            attbc = S.tile([128, K], f32)
            M10 = S.tile([C, 10], f32)
            PART0 = S.tile([C, 1], f32)
            PART1 = S.tile([C, 1], f32)
            PART2 = S.tile([C, 1], f32)
            PART3 = S.tile([C, 1], f32)
            PARTS = [PART0, PART1, PART2, PART3]
            mw = None  # allocated below once MWDT is known
            mwb = S.tile([128, 6, 64], DT, name="mwb") if conv_dt == "bf16" else None
            fold = S.tile([C, 2600], f32)
            fold2 = S.tile([C, 2600], f32)
            actout = S.tile([C, 3300], f32)
            actout2 = S.tile([C, 3300], f32)
            rs128 = S.tile([128, 1], f32)

            wpsum = PS1.tile([128, 128], f32)
            psum_b = PS1.tile([128, K], f32)

            XLv = XL.bitcast(f32) if conv_dt == "fp32r" else XL
            MWDT = mybir.dt.float32r if conv_dt == "fp32r" else f32
            mw = S.tile([128, 6, 64], MWDT, name="mw")
            # --- constants / border zeroing (DVE, all tiny) ---
            nc.vector.memset(onesrow, 0.0)
            nc.vector.memset(M10[:, 9:10], 1.0)
            # borders: host pre-pads the row gaps; only head/tail need zeroing
            nc.vector.memset(XLv[0:64, 0:132], 0.0)
            nc.vector.memset(XLv[0:64, 132 + H * WP:NELEM], 0.0)
            nc.vector.memset(XLv[64:128, 0:2], 0.0)
            nc.vector.memset(XLv[64:128, 2 + H * WP:NELEM], 0.0)

            # --- small input DMAs ---
            nc.scalar.dma_start(out=wb_sb, in_=wbk[:, :, :])
            nc.scalar.dma_start(out=cw2_sb, in_=cw2[:, :, :])
            nc.scalar.dma_start(out=convb_sb, in_=cb[:, :])

            # --- x load: fully contiguous (host pre-padded 130-wide rows) ---
            # lower copy at offset 132 (sync ring), upper row-shifted copy at
            # offset 2 (gpsimd/SWDGE ring) so the two streams overlap
            for c in range(NCHUNKS):
                a = span_elems * c
                nc.sync.dma_start(out=XL[0:64, 132 + a: 132 + a + span_elems],
                                  in_=xin[:, a: a + span_elems])
                nc.scalar.dma_start(out=XL[64:128, 2 + a: 2 + a + span_elems],
                                    in_=xin[:, a: a + span_elems])

            # --- PE warm-up (results discarded; onesrow is all-zero here) ---
            for i in range(4):
                nc.tensor.matmul(wpsum, onesrow, onesrow, start=True, stop=True)
            for c in range(NCHUNKS):
                a = 132 + span_elems * c
                for i in range(2):
                    # both operands from XL so the warm-up rides the chunk-DMA deps
                    nc.tensor.matmul(wpsum, XL[:, a:a + 128], XL[:, a + 128 * i:a + 128 * i + 128],
                                     start=True, stop=True)

            # --- attention basis sums ---
            # DVE: scalar_tensor_tensor fold (reads 2 streams/cycle) with accum_out;
            # ACT: activation-Identity with accum_out.  (tensor_tensor_reduce is
            # broken on this runtime — do not use.)
            spans = [(0, 5200, "dve", fold), (5200, 3250, "act", actout),
                     (8450, 5200, "dve", fold2), (13650, NELEM - 13650, "act", actout2)]
            if skip_attn:
                spans = []
            for idx, (a, ln, eng, obuf) in enumerate(spans):
                if spans_mode == "basic":
                    eng = "basic"
                elif spans_mode == "act":
                    eng = "act"
                if eng == "dve":
                    h = ln // 2
                    nc.vector.scalar_tensor_tensor(
                        out=obuf[:, :h], in0=XLv[0:64, a:a + h], scalar=1.0,
                        in1=XLv[0:64, a + h:a + ln], op0=Alu.mult, op1=Alu.add,
                        accum_out=PARTS[idx][:, 0:1])
                elif eng == "act":
                    nc.scalar.activation(
                        out=obuf[:, :ln], in_=XLv[0:64, a:a + ln], func=Act.Identity,
                        bias=0.0, scale=1.0, accum_out=PARTS[idx][:, 0:1])
                else:
                    nc.vector.tensor_reduce(out=PARTS[idx][:, 0:1], in_=XLv[0:64, a:a + ln],
                                            axis=Ax.X, op=Alu.add)
            if not skip_attn:
                if with_edges:
                    nc.vector.tensor_reduce(out=M10[:, 1:2], in_=XLv[0:64, 132:132 + W], axis=Ax.X, op=Alu.add)
                    nc.vector.tensor_reduce(out=M10[:, 2:3], in_=XLv[0:64, 16642:16642 + W], axis=Ax.X, op=Alu.add)
                else:
                    nc.vector.memset(M10[:, 1:3], 0.0)
                if with_cols:
                    col0 = XLv[0:64, 132:132 + WP * H].rearrange("p (r w) -> p r w", w=WP)[:, :, 0:1]
                    nc.vector.tensor_reduce(out=M10[:, 3:4], in_=col0, axis=Ax.XY, op=Alu.add)
                    col1 = XLv[0:64, 259:259 + WP * H].rearrange("p (r w) -> p r w", w=WP)[:, :, 0:1]
                    nc.vector.tensor_reduce(out=M10[:, 4:5], in_=col1, axis=Ax.XY, op=Alu.add)
                else:
                    nc.vector.memset(M10[:, 3:5], 0.0)
                if with_corners:
                    # corners: {132, 259} and {16642, 16769} via stride-127 views
                    nc.vector.tensor_copy(
                        out=M10[:, 5:7].rearrange("p (a b) -> p a b", b=1),
                        in_=XLv[0:64, 132:132 + 254].rearrange("p (a b) -> p a b", b=127)[:, :, 0:1])
                    nc.vector.tensor_copy(
                        out=M10[:, 7:9].rearrange("p (a b) -> p a b", b=1),
                        in_=XLv[0:64, 16642:16642 + 254].rearrange("p (a b) -> p a b", b=127)[:, :, 0:1])
                else:
                    nc.vector.memset(M10[:, 5:9], 0.0)

            if skip_attn or skip_logits_mm:
                nc.vector.memset(attbc, 0.25)
            else:
                # --- logits, replicated on all 128 partitions via stride-0 lhsT;
                # the 4 span partials each contract against basis-0 coefficients ---
                lhs_list = [(p[:, 0:1], 0) for p in PARTS] + \
                           [(M10[:, j:j + 1], j) for j in range(1, 10)]
                for ji, (lhs, j) in enumerate(lhs_list):
                    nc.tensor.matmul(psum_b, lhs.to_broadcast((C, 128)),
                                     cw2_sb[:, j, :], start=(ji == 0),
                                     stop=(ji == len(lhs_list) - 1))
                # softmax per partition (all partitions identical); logits are tiny,
                # so skip the max-subtraction
                nc.scalar.activation(out=att_sb, in_=psum_b, func=Act.Exp)
                nc.vector.tensor_reduce(out=rs128, in_=att_sb, axis=Ax.X, op=Alu.add)
                nc.vector.reciprocal(out=rs128, in_=rs128)
                nc.vector.tensor_scalar_mul(out=attbc, in0=att_sb, scalar1=rs128)

            # --- weight mixing: mw = conv_bank + sum_k att_k * bank_k ---
            nc.vector.scalar_tensor_tensor(
                out=mw[:, :, :], in0=wb_sb[:, 0, :].rearrange("p (m c) -> p m c", m=6),
                scalar=attbc[:, 0:1],
                in1=wb_sb[:, 4, :].rearrange("p (m c) -> p m c", m=6),
                op0=Alu.mult, op1=Alu.add)
            for k in range(1, K):
                nc.vector.scalar_tensor_tensor(
                    out=mw[:, :, :], in0=wb_sb[:, k, :].rearrange("p (m c) -> p m c", m=6),
                    scalar=attbc[:, k:k + 1], in1=mw[:, :, :],
                    op0=Alu.mult, op1=Alu.add)
            if conv_dt == "bf16":
                nc.vector.tensor_copy(out=mwb, in_=mw)
                lhs_src = mwb
            else:
                lhs_src = mw

            # --- main conv: 43 PSUM tiles x 6 accumulating matmuls ---
            r0s = [] if skip_conv else list(range(1, H + 1, ROWS_PER_TILE))
            if skip_conv:
                zst = STG.tile([64, H * 2], f32)
                nc.vector.tensor_copy(out=zst[:, 0:K], in_=attbc[0:64, :])
                nc.vector.memset(zst[:, K:H * 2], 0.0)
                for rr in range(0, H, 2):
                    nc.scalar.dma_start(out=outT[:, rr:rr + 2, :],
                                        in_=zst[:, :].rearrange("p (r w) -> p r w", w=W))
            for ti, r0 in enumerate(r0s):
                nrows = min(ROWS_PER_TILE, H + 1 - r0)
                F = WP * nrows
                pt = PS.tile([64, WP * ROWS_PER_TILE], f32, tag="cps", name=f"cps{ti}")
                pt = pt[:, :F]
                for m in range(6):
                    lhsT = lhs_src[:, m, :]
                    rhs = XL[:, WP * r0 + MM_OFFS[m] + 1: WP * r0 + MM_OFFS[m] + 1 + F]
                    nc.tensor.matmul(pt, lhsT, rhs, start=(m == 0), stop=(m == 5))
                st = STG.tile([64, WP * ROWS_PER_TILE], f32, tag="stg", name=f"stg{ti}")
                if ti % 2 == 0:
                    nc.scalar.add(out=st[:, :F], in_=pt, add=convb_sb[:, 0:1])
                else:
                    nc.vector.tensor_scalar_add(out=st[:, :F], in0=pt, scalar1=convb_sb[:, 0:1])
                src = st[:, :F].rearrange("p (r w) -> p r w", w=WP)[:, :, 1:1 + W]
                eng = nc.sync if ti % 2 == 0 else nc.scalar
                eng.dma_start(out=outT[:, r0 - 1:r0 - 1 + nrows, :], in_=src)

    nc.compile()
    return nc


def _get_nc():
    if CONV_DT not in _NC_CACHE:
        _NC_CACHE[CONV_DT] = _build_nc(CONV_DT)
    return _NC_CACHE[CONV_DT]


def _prep_inputs(x, weight, conv_w, conv_b, net0_w, net0_b, net1_w, net1_b,
                 net2_w, net2_b):
    cw2 = _make_cw2(np.asarray(net0_w, np.float32), np.asarray(net0_b, np.float32),
                    np.asarray(net1_w, np.float32), np.asarray(net1_b, np.float32),
                    np.asarray(net2_w, np.float32), np.asarray(net2_b, np.float32))
    banks = np.stack([_make_bank(np.asarray(weight, np.float32)[k]) for k in range(K)]
                     + [_make_bank(np.asarray(conv_w, np.float32))])  # (5,128,6,64)
    banks = np.ascontiguousarray(banks.reshape(5, 128, 6 * 64).transpose(1, 0, 2))
    convb = np.ascontiguousarray(np.asarray(conv_b, np.float32).reshape(C, 1))
    x = np.asarray(x, np.float32)
    xp = np.zeros((N, C, H, WP), np.float32)
    xp[:, :, :, :W] = x
    if CONV_DT == "bf16":
        import ml_dtypes
        xs = xp.astype(ml_dtypes.bfloat16)
    else:
        xs = xp
    in_maps = []
    for n in range(N):
        in_maps.append({
            "xin": np.ascontiguousarray(xs[n].reshape(C, H * WP)),
            "wbanks": banks,
            "cw2": cw2,
            "convb": convb,
        })
    return in_maps


def _run(inputs, trace=False, **kw):
    from concourse.bass_utils import run_bass_kernel_spmd
    nc = _get_nc()
    in_maps = _prep_inputs(**inputs)
    return run_bass_kernel_spmd(nc, in_maps, core_ids=list(range(N)), trace=trace, **kw)


def kernel(**inputs):
    res = _run(inputs)
    out = np.stack([res.results[n]["out"] for n in range(N)]).astype(np.float32)
    return out


# revision 25
# speedup vs baseline: 1.3409x; 1.0096x over previous
"""CondConv2d on 8 Trainium2 NeuronCores — data-parallel over batch N=8.

Per-core (one sample):
  - attention branch collapsed to 10 "basis" sums of x (total / edge rows /
    edge cols / corners / const) contracted with host-precomputed coefficients
    -> logits -> softmax -> per-sample mixed 3x3 weight, with the static
    residual conv fused in (mw = sum_k att_k W_k + conv_w; bias added at
    PSUM eviction).
  - 3x3 conv as 6 accumulating PE matmuls per PSUM tile over a 130-wide
    zero-padded layout; contraction 128 = 64 channels (lower partitions) +
    64 channels of a row-shifted copy (upper partitions), pairing taps
    (-1,w)+(0,w) per matmul.
"""
import os
import numpy as np

N, C, H, W = 8, 64, 128, 128
K = 4
WP = W + 2                 # padded row width (130)
NELEM = WP * WP + 2        # per-partition x buffer length (16902); xpadflat at elem offset 1
ROWS_PER_TILE = 3          # output rows per PSUM tile (free dim 390 <= 512)
NCHUNKS = 4                # x load/reduce chunks (128 rows / NCHUNKS each)

CONV_DT = os.environ.get("KCONV_DT", "fp32r")   # "fp32" | "fp32r" | "bf16"

MM_TAPS = [((-1, -1), (0, -1)), ((-1, 0), (0, 0)), ((-1, 1), (0, 1)),
           ((1, -1), None), ((1, 0), None), ((1, 1), None)]
MM_OFFS = [130 * L[0] + L[1] for L, _ in MM_TAPS]


# ----------------------------------------------------------------------------
# host-side prep
# ----------------------------------------------------------------------------
def _make_cw2(net0_w, net0_b, net1_w, net1_b, net2_w, net2_b):
    """CW2[c, b, k]: logits[k] = sum_{c,b} CW2[c,b,k] * basis[c,b].
    basis: 0=total, 1=row0, 2=row127, 3=col0, 4=col127,
           5..8=corners (00,0W,H0,HW), 9=const 1."""
    cw = np.zeros((C, 10, K), np.float64)
    scale = 1.0 / (C * H * W)
    for w_net, pads in ((net0_w, (0, 0, 0)), (net1_w, (1, 1, 1)), (net2_w, (2, 1, 1))):
        Kk, _, kd, kh, kw = w_net.shape
        pd, ph, pw = pads
        for i in range(kd):
            clo, chi = max(0, i - pd), min(C - 1, C - 1 + i - pd)
            cmask = np.zeros(C)
            cmask[clo:chi + 1] = 1.0
            for j in range(kh):
                hlo, hhi = max(0, j - ph), min(H - 1, H - 1 + j - ph)
                dropA = 0 if hlo == 1 else (127 if hhi == H - 2 else None)
                for l in range(kw):
                    wlo, whi = max(0, l - pw), min(W - 1, W - 1 + l - pw)
                    dropB = 0 if wlo == 1 else (127 if whi == W - 2 else None)
                    v = np.zeros(10)
                    v[0] = 1.0
                    if dropA == 0: v[1] = -1.0
                    if dropA == 127: v[2] = -1.0
                    if dropB == 0: v[3] = -1.0
                    if dropB == 127: v[4] = -1.0
                    if dropA is not None and dropB is not None:
                        v[{(0, 0): 5, (0, 127): 6, (127, 0): 7, (127, 127): 8}[(dropA, dropB)]] = 1.0
                    for k in range(Kk):
                        cw[:, :, k] += w_net[k, 0, i, j, l] * scale * np.outer(cmask, v)
    btot = (net0_b + net1_b + net2_b).astype(np.float64)
    cw[:, 9, :] += btot[None, :] / C
    return np.ascontiguousarray(cw.astype(np.float32))


def _make_bank(Wt):
    """Wt (co, ci, 3, 3) -> (128, 6, 64): [p=ci(lo)/64+ci(hi), mm, co]."""
    bank = np.zeros((128, 6, 64), np.float32)
    for m, (L, Hh) in enumerate(MM_TAPS):
        bank[:64, m, :] = Wt[:, :, 1 + L[0], 1 + L[1]].T
        if Hh is not None:
            bank[64:, m, :] = Wt[:, :, 1 + Hh[0], 1 + Hh[1]].T
    return bank


# ----------------------------------------------------------------------------
# device program
# ----------------------------------------------------------------------------
_NC_CACHE = {}


def _build_nc(conv_dt, skip_attn=False, skip_conv=False, skip_logits_mm=False,
              spans_mode="mixed", with_edges=True, with_cols=True, with_corners=True):
    import concourse.bacc as bacc
    import concourse.tile as tile
    from concourse import mybir

    f32 = mybir.dt.float32
    if conv_dt == "bf16":
        DT = mybir.dt.bfloat16
    elif conv_dt == "fp32r":
        DT = mybir.dt.float32r
    else:
        DT = f32
    Alu = mybir.AluOpType
    Ax = mybir.AxisListType
    Act = mybir.ActivationFunctionType

    nc = bacc.Bacc("TRN2", target_bir_lowering=False, debug=False,
                   enable_asserts=False, num_devices=N)
    xin = nc.dram_tensor("xin", [C, H * WP], DT, kind="ExternalInput")
    wbk = nc.dram_tensor("wbanks", [128, 5, 6 * 64], f32, kind="ExternalInput")
    cw2 = nc.dram_tensor("cw2", [C, 10, K], f32, kind="ExternalInput")
    cb = nc.dram_tensor("convb", [C, 1], f32, kind="ExternalInput")
    outT = nc.dram_tensor("out", [C, H, W], f32, kind="ExternalOutput")

    rows_per_chunk = H // NCHUNKS                      # 32
    span_elems = WP * rows_per_chunk                   # 4160

    with tile.TileContext(nc) as tc:
        with tc.tile_pool(name="singles", bufs=1) as S, \
             tc.tile_pool(name="stage", bufs=4) as STG, \
             tc.tile_pool(name="cpsum", bufs=4, space="PSUM") as PS, \
             tc.tile_pool(name="spsum", bufs=1, space="PSUM") as PS1:

            XL = S.tile([128, NELEM], DT)
            wb_sb = S.tile([128, 5, 6 * 64], f32)
            cw2_sb = S.tile([C, 10, K], f32)
            convb_sb = S.tile([C, 1], f32)
            onesrow = S.tile([128, 128], f32)
            att_sb = S.tile([128, K], f32)
            attbc = S.tile([128, K], f32)
            M10 = S.tile([C, 10], f32)
            PART0 = S.tile([C, 1], f32)
            PART1 = S.tile([C, 1], f32)
            PART2 = S.tile([C, 1], f32)
            PART3 = S.tile([C, 1], f32)
            PARTS = [PART0, PART1, PART2, PART3]
            mw = None  # allocated below once MWDT is known
            mwb = S.tile([128, 6, 64], DT, name="mwb") if conv_dt == "bf16" else None
            fold = S.tile([C, 2600], f32)
            fold2 = S.tile([C, 2600], f32)
            actout = S.tile([C, 3300], f32)
            actout2 = S.tile([C, 3300], f32)
            rs128 = S.tile([128, 1], f32)

            wpsum = PS1.tile([128, 128], f32)
            psum_b = PS1.tile([128, K], f32)

            XLv = XL.bitcast(f32) if conv_dt == "fp32r" else XL
            MWDT = mybir.dt.float32r if conv_dt == "fp32r" else f32
            mw = S.tile([128, 6, 64], MWDT, name="mw")
            # --- constants / border zeroing (DVE, all tiny) ---
            nc.vector.memset(onesrow, 0.0)
            nc.vector.memset(M10[:, 9:10], 1.0)
            # borders: host pre-pads the row gaps; only head/tail need zeroing
            nc.vector.memset(XLv[0:64, 0:132], 0.0)
            nc.vector.memset(XLv[0:64, 132 + H * WP:NELEM], 0.0)
            nc.vector.memset(XLv[64:128, 0:2], 0.0)
            nc.vector.memset(XLv[64:128, 2 + H * WP:NELEM], 0.0)

            # --- small input DMAs ---
            nc.scalar.dma_start(out=wb_sb, in_=wbk[:, :, :])
            nc.scalar.dma_start(out=cw2_sb, in_=cw2[:, :, :])
            nc.scalar.dma_start(out=convb_sb, in_=cb[:, :])

            # --- x load: fully contiguous (host pre-padded 130-wide rows) ---
            # lower copy at offset 132 (sync ring), upper row-shifted copy at
            # offset 2 (gpsimd/SWDGE ring) so the two streams overlap
            for c in range(NCHUNKS):
                a = span_elems * c
                nc.sync.dma_start(out=XL[0:64, 132 + a: 132 + a + span_elems],
                                  in_=xin[:, a: a + span_elems])
                nc.sync.dma_start(out=XL[64:128, 2 + a: 2 + a + span_elems],
                                    in_=xin[:, a: a + span_elems])

            # --- PE warm-up (results discarded; onesrow is all-zero here) ---
            for i in range(8):
                nc.tensor.matmul(wpsum, onesrow, onesrow, start=True, stop=True)

            # --- attention basis sums ---
            # DVE: scalar_tensor_tensor fold (reads 2 streams/cycle) with accum_out;
            # ACT: activation-Identity with accum_out.  (tensor_tensor_reduce is
            # broken on this runtime — do not use.)
            spans = [(0, 5200, "dve", fold), (5200, 3250, "act", actout),
                     (8450, 5200, "dve", fold2), (13650, NELEM - 13650, "act", actout2)]
            if skip_attn:
                spans = []
            for idx, (a, ln, eng, obuf) in enumerate(spans):
                if spans_mode == "basic":
                    eng = "basic"
                elif spans_mode == "act":
                    eng = "act"
                if eng == "dve":
                    h = ln // 2
                    nc.vector.scalar_tensor_tensor(
                        out=obuf[:, :h], in0=XLv[0:64, a:a + h], scalar=1.0,
                        in1=XLv[0:64, a + h:a + ln], op0=Alu.mult, op1=Alu.add,
                        accum_out=PARTS[idx][:, 0:1])
                elif eng == "act":
                    nc.scalar.activation(
                        out=obuf[:, :ln], in_=XLv[0:64, a:a + ln], func=Act.Identity,
                        bias=0.0, scale=1.0, accum_out=PARTS[idx][:, 0:1])
                else:
                    nc.vector.tensor_reduce(out=PARTS[idx][:, 0:1], in_=XLv[0:64, a:a + ln],
                                            axis=Ax.X, op=Alu.add)
            if not skip_attn:
                if with_edges:
                    nc.vector.tensor_reduce(out=M10[:, 1:2], in_=XLv[0:64, 132:132 + W], axis=Ax.X, op=Alu.add)
                    nc.vector.tensor_reduce(out=M10[:, 2:3], in_=XLv[0:64, 16642:16642 + W], axis=Ax.X, op=Alu.add)
                else:
                    nc.vector.memset(M10[:, 1:3], 0.0)
                if with_cols:
                    col0 = XLv[0:64, 132:132 + WP * H].rearrange("p (r w) -> p r w", w=WP)[:, :, 0:1]
                    nc.vector.tensor_reduce(out=M10[:, 3:4], in_=col0, axis=Ax.XY, op=Alu.add)
                    col1 = XLv[0:64, 259:259 + WP * H].rearrange("p (r w) -> p r w", w=WP)[:, :, 0:1]
                    nc.vector.tensor_reduce(out=M10[:, 4:5], in_=col1, axis=Ax.XY, op=Alu.add)
                else:
                    nc.vector.memset(M10[:, 3:5], 0.0)
                if with_corners:
                    # corners: {132, 259} and {16642, 16769} via stride-127 views
                    nc.vector.tensor_copy(
                        out=M10[:, 5:7].rearrange("p (a b) -> p a b", b=1),
                        in_=XLv[0:64, 132:132 + 254].rearrange("p (a b) -> p a b", b=127)[:, :, 0:1])
                    nc.vector.tensor_copy(
                        out=M10[:, 7:9].rearrange("p (a b) -> p a b", b=1),
                        in_=XLv[0:64, 16642:16642 + 254].rearrange("p (a b) -> p a b", b=127)[:, :, 0:1])
                else:
                    nc.vector.memset(M10[:, 5:9], 0.0)

            if skip_attn or skip_logits_mm:
                nc.vector.memset(attbc, 0.25)
            else:
                # --- logits, replicated on all 128 partitions via stride-0 lhsT;
                # the 4 span partials each contract against basis-0 coefficients ---
                lhs_list = [(p[:, 0:1], 0) for p in PARTS] + \
                           [(M10[:, j:j + 1], j) for j in range(1, 10)]
                for ji, (lhs, j) in enumerate(lhs_list):
                    nc.tensor.matmul(psum_b, lhs.to_broadcast((C, 128)),
                                     cw2_sb[:, j, :], start=(ji == 0),
                                     stop=(ji == len(lhs_list) - 1))
                # softmax per partition (all partitions identical); logits are tiny,
                # so skip the max-subtraction
                nc.scalar.activation(out=att_sb, in_=psum_b, func=Act.Exp)
                nc.vector.tensor_reduce(out=rs128, in_=att_sb, axis=Ax.X, op=Alu.add)
                nc.vector.reciprocal(out=rs128, in_=rs128)
                nc.vector.tensor_scalar_mul(out=attbc, in0=att_sb, scalar1=rs128)

            # --- weight mixing: mw = conv_bank + sum_k att_k * bank_k ---
            nc.vector.scalar_tensor_tensor(
                out=mw[:, :, :], in0=wb_sb[:, 0, :].rearrange("p (m c) -> p m c", m=6),
                scalar=attbc[:, 0:1],
                in1=wb_sb[:, 4, :].rearrange("p (m c) -> p m c", m=6),
                op0=Alu.mult, op1=Alu.add)
            for k in range(1, K):
                nc.vector.scalar_tensor_tensor(
                    out=mw[:, :, :], in0=wb_sb[:, k, :].rearrange("p (m c) -> p m c", m=6),
                    scalar=attbc[:, k:k + 1], in1=mw[:, :, :],
                    op0=Alu.mult, op1=Alu.add)
            if conv_dt == "bf16":
                nc.vector.tensor_copy(out=mwb, in_=mw)
                lhs_src = mwb
            else:
                lhs_src = mw

            # --- main conv: 43 PSUM tiles x 6 accumulating matmuls ---
            r0s = [] if skip_conv else list(range(1, H + 1, ROWS_PER_TILE))
            if skip_conv:
                zst = STG.tile([64, H * 2], f32)
                nc.vector.tensor_copy(out=zst[:, 0:K], in_=attbc[0:64, :])
                nc.vector.memset(zst[:, K:H * 2], 0.0)
                for rr in range(0, H, 2):
                    nc.scalar.dma_start(out=outT[:, rr:rr + 2, :],
                                        in_=zst[:, :].rearrange("p (r w) -> p r w", w=W))
            for ti, r0 in enumerate(r0s):
                nrows = min(ROWS_PER_TILE, H + 1 - r0)
                F = WP * nrows
                pt = PS.tile([64, WP * ROWS_PER_TILE], f32, tag="cps", name=f"cps{ti}")
                pt = pt[:, :F]
                for m in range(6):
                    lhsT = lhs_src[:, m, :]
                    rhs = XL[:, WP * r0 + MM_OFFS[m] + 1: WP * r0 + MM_OFFS[m] + 1 + F]
                    nc.tensor.matmul(pt, lhsT, rhs, start=(m == 0), stop=(m == 5))
                st = STG.tile([64, WP * ROWS_PER_TILE], f32, tag="stg", name=f"stg{ti}")
                if ti % 2 == 0:
                    nc.scalar.add(out=st[:, :F], in_=pt, add=convb_sb[:, 0:1])
                else:
                    nc.vector.tensor_scalar_add(out=st[:, :F], in0=pt, scalar1=convb_sb[:, 0:1])
                src = st[:, :F].rearrange("p (r w) -> p r w", w=WP)[:, :, 1:1 + W]
                eng = nc.sync if ti % 2 == 0 else nc.scalar
                eng.dma_start(out=outT[:, r0 - 1:r0 - 1 + nrows, :], in_=src)

    nc.compile()
    return nc


def _get_nc():
    if CONV_DT not in _NC_CACHE:
        _NC_CACHE[CONV_DT] = _build_nc(CONV_DT)
    return _NC_CACHE[CONV_DT]


def _prep_inputs(x, weight, conv_w, conv_b, net0_w, net0_b, net1_w, net1_b,
                 net2_w, net2_b):
    cw2 = _make_cw2(np.asarray(net0_w, np.float32), np.asarray(net0_b, np.float32),
                    np.asarray(net1_w, np.float32), np.asarray(net1_b, np.float32),
                    np.asarray(net2_w, np.float32), np.asarray(net2_b, np.float32))
    banks = np.stack([_make_bank(np.asarray(weight, np.float32)[k]) for k in range(K)]
                     + [_make_bank(np.asarray(conv_w, np.float32))])  # (5,128,6,64)
    banks = np.ascontiguousarray(banks.reshape(5, 128, 6 * 64).transpose(1, 0, 2))
    convb = np.ascontiguousarray(np.asarray(conv_b, np.float32).reshape(C, 1))
    x = np.asarray(x, np.float32)
    xp = np.zeros((N, C, H, WP), np.float32)
    xp[:, :, :, :W] = x
    if CONV_DT == "bf16":
        import ml_dtypes
        xs = xp.astype(ml_dtypes.bfloat16)
    else:
        xs = xp
    in_maps = []
    for n in range(N):
        in_maps.append({
            "xin": np.ascontiguousarray(xs[n].reshape(C, H * WP)),
            "wbanks": banks,
            "cw2": cw2,
            "convb": convb,
        })
    return in_maps


def _run(inputs, trace=False, **kw):
    from concourse.bass_utils import run_bass_kernel_spmd
    nc = _get_nc()
    in_maps = _prep_inputs(**inputs)
    return run_bass_kernel_spmd(nc, in_maps, core_ids=list(range(N)), trace=trace, **kw)


def kernel(**inputs):
    res = _run(inputs)
    out = np.stack([res.results[n]["out"] for n in range(N)]).astype(np.float32)
    return out


# revision 27
# speedup vs baseline: 1.3686x; 1.0207x over previous
"""CondConv2d on 8 Trainium2 NeuronCores — data-parallel over batch N=8.

Per-core (one sample):
  - attention branch collapsed to 10 "basis" sums of x (total / edge rows /
    edge cols / corners / const) contracted with host-precomputed coefficients
    -> logits -> softmax -> per-sample mixed 3x3 weight, with the static
    residual conv fused in (mw = sum_k att_k W_k + conv_w; bias added at
    PSUM eviction).
  - 3x3 conv as 6 accumulating PE matmuls per PSUM tile over a 130-wide
    zero-padded layout; contraction 128 = 64 channels (lower partitions) +
    64 channels of a row-shifted copy (upper partitions), pairing taps
    (-1,w)+(0,w) per matmul.
"""
import os
import numpy as np

N, C, H, W = 8, 64, 128, 128
K = 4
WP = W + 2                 # padded row width (130)
NELEM = WP * WP + 2        # per-partition x buffer length (16902); xpadflat at elem offset 1
ROWS_PER_TILE = 3          # output rows per PSUM tile (free dim 390 <= 512)
NCHUNKS = 2                # x load/reduce chunks (128 rows / NCHUNKS each)

CONV_DT = os.environ.get("KCONV_DT", "fp32r")   # "fp32" | "fp32r" | "bf16"

MM_TAPS = [((-1, -1), (0, -1)), ((-1, 0), (0, 0)), ((-1, 1), (0, 1)),
           ((1, -1), None), ((1, 0), None), ((1, 1), None)]
MM_OFFS = [130 * L[0] + L[1] for L, _ in MM_TAPS]


# ----------------------------------------------------------------------------
# host-side prep
# ----------------------------------------------------------------------------
def _make_cw2(net0_w, net0_b, net1_w, net1_b, net2_w, net2_b):
    """CW2[c, b, k]: logits[k] = sum_{c,b} CW2[c,b,k] * basis[c,b].
    basis: 0=total, 1=row0, 2=row127, 3=col0, 4=col127,
           5..8=corners (00,0W,H0,HW), 9=const 1."""
    cw = np.zeros((C, 10, K), np.float64)
    scale = 1.0 / (C * H * W)
    for w_net, pads in ((net0_w, (0, 0, 0)), (net1_w, (1, 1, 1)), (net2_w, (2, 1, 1))):
        Kk, _, kd, kh, kw = w_net.shape
        pd, ph, pw = pads
        for i in range(kd):
            clo, chi = max(0, i - pd), min(C - 1, C - 1 + i - pd)
            cmask = np.zeros(C)
            cmask[clo:chi + 1] = 1.0
            for j in range(kh):
                hlo, hhi = max(0, j - ph), min(H - 1, H - 1 + j - ph)
                dropA = 0 if hlo == 1 else (127 if hhi == H - 2 else None)
                for l in range(kw):
                    wlo, whi = max(0, l - pw), min(W - 1, W - 1 + l - pw)
                    dropB = 0 if wlo == 1 else (127 if whi == W - 2 else None)
                    v = np.zeros(10)
                    v[0] = 1.0
                    if dropA == 0: v[1] = -1.0
                    if dropA == 127: v[2] = -1.0
                    if dropB == 0: v[3] = -1.0
                    if dropB == 127: v[4] = -1.0
                    if dropA is not None and dropB is not None:
                        v[{(0, 0): 5, (0, 127): 6, (127, 0): 7, (127, 127): 8}[(dropA, dropB)]] = 1.0
                    for k in range(Kk):
                        cw[:, :, k] += w_net[k, 0, i, j, l] * scale * np.outer(cmask, v)
    btot = (net0_b + net1_b + net2_b).astype(np.float64)
    cw[:, 9, :] += btot[None, :] / C
    return np.ascontiguousarray(cw.astype(np.float32))


def _make_bank(Wt):
    """Wt (co, ci, 3, 3) -> (128, 6, 64): [p=ci(lo)/64+ci(hi), mm, co]."""
    bank = np.zeros((128, 6, 64), np.float32)
    for m, (L, Hh) in enumerate(MM_TAPS):
        bank[:64, m, :] = Wt[:, :, 1 + L[0], 1 + L[1]].T
        if Hh is not None:
            bank[64:, m, :] = Wt[:, :, 1 + Hh[0], 1 + Hh[1]].T
    return bank


# ----------------------------------------------------------------------------
# device program
# ----------------------------------------------------------------------------
_NC_CACHE = {}


def _build_nc(conv_dt, skip_attn=False, skip_conv=False, skip_logits_mm=False,
              spans_mode="mixed", with_edges=True, with_cols=True, with_corners=True):
    import concourse.bacc as bacc
    import concourse.tile as tile
    from concourse import mybir

    f32 = mybir.dt.float32
    if conv_dt == "bf16":
        DT = mybir.dt.bfloat16
    elif conv_dt == "fp32r":
        DT = mybir.dt.float32r
    else:
        DT = f32
    Alu = mybir.AluOpType
    Ax = mybir.AxisListType
    Act = mybir.ActivationFunctionType

    nc = bacc.Bacc("TRN2", target_bir_lowering=False, debug=False,
                   enable_asserts=False, num_devices=N)
    xin = nc.dram_tensor("xin", [C, H * WP], DT, kind="ExternalInput")
    wbk = nc.dram_tensor("wbanks", [128, 5, 6 * 64], f32, kind="ExternalInput")
    cw2 = nc.dram_tensor("cw2", [C, 10, K], f32, kind="ExternalInput")
    cb = nc.dram_tensor("convb", [C, 1], f32, kind="ExternalInput")
    outT = nc.dram_tensor("out", [C, H, W], f32, kind="ExternalOutput")

    rows_per_chunk = H // NCHUNKS                      # 32
    span_elems = WP * rows_per_chunk                   # 4160

    with tile.TileContext(nc) as tc:
        with tc.tile_pool(name="singles", bufs=1) as S, \
             tc.tile_pool(name="stage", bufs=4) as STG, \
             tc.tile_pool(name="cpsum", bufs=4, space="PSUM") as PS, \
             tc.tile_pool(name="spsum", bufs=1, space="PSUM") as PS1:

            XL = S.tile([128, NELEM], DT)
            wb_sb = S.tile([128, 5, 6 * 64], f32)
            cw2_sb = S.tile([C, 10, K], f32)
            convb_sb = S.tile([C, 1], f32)
            onesrow = S.tile([128, 128], f32)
            att_sb = S.tile([128, K], f32)
            attbc = S.tile([128, K], f32)
            M10 = S.tile([C, 10], f32)
            PART0 = S.tile([C, 1], f32)
            PART1 = S.tile([C, 1], f32)
            PART2 = S.tile([C, 1], f32)
            PART3 = S.tile([C, 1], f32)
            PARTS = [PART0, PART1, PART2, PART3]
            mw = None  # allocated below once MWDT is known
            mwb = S.tile([128, 6, 64], DT, name="mwb") if conv_dt == "bf16" else None
            fold = S.tile([C, 2700], f32)
            fold2 = S.tile([C, 2700], f32)
            actout = S.tile([C, 3300], f32)
            actout2 = S.tile([C, 3300], f32)
            rs128 = S.tile([128, 1], f32)

            wpsum = PS1.tile([128, 512], f32)
            psum_b = PS1.tile([128, K], f32)

            XLv = XL.bitcast(f32) if conv_dt == "fp32r" else XL
            MWDT = mybir.dt.float32r if conv_dt == "fp32r" else f32
            mw = S.tile([128, 6, 64], MWDT, name="mw")
            # --- constants / border zeroing (DVE, all tiny) ---
            nc.vector.memset(onesrow, 0.0)
            nc.vector.memset(M10[:, 9:10], 1.0)
            # borders: host pre-pads the row gaps; only head/tail need zeroing
            nc.vector.memset(XLv[0:64, 0:132], 0.0)
            nc.vector.memset(XLv[0:64, 132 + H * WP:NELEM], 0.0)
            nc.vector.memset(XLv[64:128, 0:2], 0.0)
            nc.vector.memset(XLv[64:128, 2 + H * WP:NELEM], 0.0)

            # --- small input DMAs ---
            nc.scalar.dma_start(out=wb_sb, in_=wbk[:, :, :])
            nc.scalar.dma_start(out=cw2_sb, in_=cw2[:, :, :])
            nc.scalar.dma_start(out=convb_sb, in_=cb[:, :])

            # --- x load: fully contiguous (host pre-padded 130-wide rows) ---
            # lower copies first (the attention reductions read only the lower
            # half); the row-shifted upper copies are only needed by the conv
            for c in range(NCHUNKS):
                a = span_elems * c
                nc.sync.dma_start(out=XL[0:64, 132 + a: 132 + a + span_elems],
                                  in_=xin[:, a: a + span_elems])
            for c in range(NCHUNKS):
                a = span_elems * c
                nc.sync.dma_start(out=XL[64:128, 2 + a: 2 + a + span_elems],
                                  in_=xin[:, a: a + span_elems])

            # --- PE warm-up (results discarded; onesrow is all-zero here) ---
            for i in range(8):
                nc.tensor.matmul(wpsum[:, 0:128], onesrow, onesrow, start=True, stop=True)

            # --- attention basis sums ---
            # DVE: scalar_tensor_tensor fold (reads 2 streams/cycle) with accum_out;
            # ACT: activation-Identity with accum_out.  (tensor_tensor_reduce is
            # broken on this runtime — do not use.)
            spans = [(0, 5300, "dve", fold), (5300, 3152, "act", actout),
                     (8452, 5248, "dve", fold2), (13700, NELEM - 13700, "act", actout2)]
            if skip_attn:
                spans = []
            # row-0 edge sum can start as soon as chunk 0 lands
            if not skip_attn:
                nc.vector.tensor_reduce(out=M10[:, 1:2], in_=XLv[0:64, 132:132 + W], axis=Ax.X, op=Alu.add)
            for idx, (a, ln, eng, obuf) in enumerate(spans):
                if idx == 2 and not skip_attn:
                    # chunk-1-dependent small reductions, queued on DVE before the
                    # second big span so the logits matmuls unblock early
                    if with_edges:
                        nc.vector.tensor_reduce(out=M10[:, 2:3], in_=XLv[0:64, 16642:16642 + W], axis=Ax.X, op=Alu.add)
                    if with_cols:
                        col0 = XLv[0:64, 132:132 + WP * H].rearrange("p (r w) -> p r w", w=WP)[:, :, 0:1]
                        nc.vector.tensor_reduce(out=M10[:, 3:4], in_=col0, axis=Ax.XY, op=Alu.add)
                        col1 = XLv[0:64, 259:259 + WP * H].rearrange("p (r w) -> p r w", w=WP)[:, :, 0:1]
                        nc.vector.tensor_reduce(out=M10[:, 4:5], in_=col1, axis=Ax.XY, op=Alu.add)
                    if with_corners:
                        nc.vector.tensor_copy(
                            out=M10[:, 5:7].rearrange("p (a b) -> p a b", b=1),
                            in_=XLv[0:64, 132:132 + 254].rearrange("p (a b) -> p a b", b=127)[:, :, 0:1])
                        nc.vector.tensor_copy(
                            out=M10[:, 7:9].rearrange("p (a b) -> p a b", b=1),
                            in_=XLv[0:64, 16642:16642 + 254].rearrange("p (a b) -> p a b", b=127)[:, :, 0:1])
                if spans_mode == "basic":
                    eng = "basic"
                elif spans_mode == "act":
                    eng = "act"
                if eng == "dve":
                    h = ln // 2
                    nc.vector.scalar_tensor_tensor(
                        out=obuf[:, :h], in0=XLv[0:64, a:a + h], scalar=1.0,
                        in1=XLv[0:64, a + h:a + ln], op0=Alu.mult, op1=Alu.add,
                        accum_out=PARTS[idx][:, 0:1])
                elif eng == "act":
                    nc.scalar.activation(
                        out=obuf[:, :ln], in_=XLv[0:64, a:a + ln], func=Act.Identity,
                        bias=0.0, scale=1.0, accum_out=PARTS[idx][:, 0:1])
                else:
                    nc.vector.tensor_reduce(out=PARTS[idx][:, 0:1], in_=XLv[0:64, a:a + ln],
                                            axis=Ax.X, op=Alu.add)
            if skip_attn or skip_logits_mm:
                nc.vector.memset(attbc, 0.25)
            else:
                # keep the PE clock warm through the logits matmuls: dummy
                # matmuls gated on the first span's scratch output
                for i in range(5):
                    nc.tensor.matmul(wpsum, onesrow[0:64, :], fold[:, i * 512:(i + 1) * 512],
                                     start=True, stop=True)
                # --- logits, replicated on all 128 partitions via stride-0 lhsT;
                # the 4 span partials each contract against basis-0 coefficients,
                # ordered by when their producers finish ---
                lhs_list = [(PARTS[0][:, 0:1], 0)] + \
                           [(M10[:, j:j + 1], j) for j in range(1, 10)] + \
                           [(PARTS[i][:, 0:1], 0) for i in (1, 2, 3)]
                for ji, (lhs, j) in enumerate(lhs_list):
                    nc.tensor.matmul(psum_b, lhs.to_broadcast((C, 128)),
                                     cw2_sb[:, j, :], start=(ji == 0),
                                     stop=(ji == len(lhs_list) - 1))
                # softmax per partition (all partitions identical); logits are tiny,
                # so skip the max-subtraction
                nc.scalar.activation(out=att_sb, in_=psum_b, func=Act.Exp)
                nc.vector.tensor_reduce(out=rs128, in_=att_sb, axis=Ax.X, op=Alu.add)
                nc.vector.reciprocal(out=rs128, in_=rs128)
                nc.vector.tensor_scalar_mul(out=attbc, in0=att_sb, scalar1=rs128)

            # --- weight mixing: mw = conv_bank + sum_k att_k * bank_k ---
            nc.vector.scalar_tensor_tensor(
                out=mw[:, :, :], in0=wb_sb[:, 0, :].rearrange("p (m c) -> p m c", m=6),
                scalar=attbc[:, 0:1],
                in1=wb_sb[:, 4, :].rearrange("p (m c) -> p m c", m=6),
                op0=Alu.mult, op1=Alu.add)
            for k in range(1, K):
                nc.vector.scalar_tensor_tensor(
                    out=mw[:, :, :], in0=wb_sb[:, k, :].rearrange("p (m c) -> p m c", m=6),
                    scalar=attbc[:, k:k + 1], in1=mw[:, :, :],
                    op0=Alu.mult, op1=Alu.add)
            if conv_dt == "bf16":
                nc.vector.tensor_copy(out=mwb, in_=mw)
                lhs_src = mwb
            else:
                lhs_src = mw

            # --- main conv: 43 PSUM tiles x 6 accumulating matmuls ---
            r0s = [] if skip_conv else list(range(1, H + 1, ROWS_PER_TILE))
            if skip_conv:
                zst = STG.tile([64, H * 2], f32)
                nc.vector.tensor_copy(out=zst[:, 0:K], in_=attbc[0:64, :])
                nc.vector.memset(zst[:, K:H * 2], 0.0)
                for rr in range(0, H, 2):
                    nc.scalar.dma_start(out=outT[:, rr:rr + 2, :],
                                        in_=zst[:, :].rearrange("p (r w) -> p r w", w=W))
            for ti, r0 in enumerate(r0s):
                nrows = min(ROWS_PER_TILE, H + 1 - r0)
                F = WP * nrows
                pt = PS.tile([64, WP * ROWS_PER_TILE], f32, tag="cps", name=f"cps{ti}")
                pt = pt[:, :F]
                for m in range(6):
                    lhsT = lhs_src[:, m, :]
                    rhs = XL[:, WP * r0 + MM_OFFS[m] + 1: WP * r0 + MM_OFFS[m] + 1 + F]
                    nc.tensor.matmul(pt, lhsT, rhs, start=(m == 0), stop=(m == 5))
                st = STG.tile([64, WP * ROWS_PER_TILE], f32, tag="stg", name=f"stg{ti}")
                if ti % 2 == 0:
                    nc.scalar.add(out=st[:, :F], in_=pt, add=convb_sb[:, 0:1])
                else:
                    nc.vector.tensor_scalar_add(out=st[:, :F], in0=pt, scalar1=convb_sb[:, 0:1])
                src = st[:, :F].rearrange("p (r w) -> p r w", w=WP)[:, :, 1:1 + W]
                eng = nc.sync if ti % 2 == 0 else nc.scalar
                eng.dma_start(out=outT[:, r0 - 1:r0 - 1 + nrows, :], in_=src)

    nc.compile()
    return nc


def _get_nc():
    if CONV_DT not in _NC_CACHE:
        _NC_CACHE[CONV_DT] = _build_nc(CONV_DT)
    return _NC_CACHE[CONV_DT]


def _prep_inputs(x, weight, conv_w, conv_b, net0_w, net0_b, net1_w, net1_b,
                 net2_w, net2_b):
    cw2 = _make_cw2(np.asarray(net0_w, np.float32), np.asarray(net0_b, np.float32),
                    np.asarray(net1_w, np.float32), np.asarray(net1_b, np.float32),
                    np.asarray(net2_w, np.float32), np.asarray(net2_b, np.float32))
    banks = np.stack([_make_bank(np.asarray(weight, np.float32)[k]) for k in range(K)]
                     + [_make_bank(np.asarray(conv_w, np.float32))])  # (5,128,6,64)
    banks = np.ascontiguousarray(banks.reshape(5, 128, 6 * 64).transpose(1, 0, 2))
    convb = np.ascontiguousarray(np.asarray(conv_b, np.float32).reshape(C, 1))
    x = np.asarray(x, np.float32)
    xp = np.zeros((N, C, H, WP), np.float32)
    xp[:, :, :, :W] = x
    if CONV_DT == "bf16":
        import ml_dtypes
        xs = xp.astype(ml_dtypes.bfloat16)
    else:
        xs = xp
    in_maps = []
    for n in range(N):
        in_maps.append({
            "xin": np.ascontiguousarray(xs[n].reshape(C, H * WP)),
            "wbanks": banks,
            "cw2": cw2,
            "convb": convb,
        })
    return in_maps


def _run(inputs, trace=False, **kw):
    from concourse.bass_utils import run_bass_kernel_spmd
    nc = _get_nc()
    in_maps = _prep_inputs(**inputs)
    return run_bass_kernel_spmd(nc, in_maps, core_ids=list(range(N)), trace=trace, **kw)


def kernel(**inputs):
    res = _run(inputs)
    out = np.stack([res.results[n]["out"] for n in range(N)]).astype(np.float32)
    return out


# revision 28
# speedup vs baseline: 1.3949x; 1.0192x over previous
"""CondConv2d on 8 Trainium2 NeuronCores — data-parallel over batch N=8.

Per-core (one sample):
  - The attention branch (three global-mean-pooled conv3ds) collapses to a
    linear function of 13 "basis" sums of x: 4 partial totals, edge rows/cols,
    corners, and a constant.  Basis sums are computed with fused
    accumulate-reductions split across the Vector and Scalar engines, the
    (channel x basis) x coefficient contraction runs as 4 tiny fused DVE ops +
    one 64->128-broadcast matmul, then softmax and per-sample weight mixing
    (the static residual conv is fused in: mw = sum_k att_k W_k + conv_w;
    conv bias is added at PSUM eviction).
  - The 3x3 conv runs as 6 accumulating PE matmuls per PSUM tile over a
    130-wide zero-padded layout; contraction 128 = 64 channels (lower
    partitions) + 64 channels of a row-shifted copy (upper partitions),
    pairing taps (-1,w)+(0,w) per matmul.  The row-shifted copy is produced
    by an on-chip SBUF->SBUF DMA so x is read from HBM only once.
"""
import os
import numpy as np

N, C, H, W = 8, 64, 128, 128
K = 4
WP = W + 2                 # padded row width (130)
NELEM = WP * WP + 2        # per-partition x buffer length (16902)
ROWS_PER_TILE = 3          # output rows per PSUM tile (free dim 390 <= 512)
NCHUNKS = 2                # x load chunks

CONV_DT = os.environ.get("KCONV_DT", "fp32r")   # "fp32" | "fp32r" | "bf16"

MM_TAPS = [((-1, -1), (0, -1)), ((-1, 0), (0, 0)), ((-1, 1), (0, 1)),
           ((1, -1), None), ((1, 0), None), ((1, 1), None)]
MM_OFFS = [130 * L[0] + L[1] for L, _ in MM_TAPS]


# ----------------------------------------------------------------------------
# host-side prep
# ----------------------------------------------------------------------------
def _make_cw2(net0_w, net0_b, net1_w, net1_b, net2_w, net2_b):
    """CW2[c, b, k]: logits[k] = sum_{c,b} CW2[c,b,k] * basis[c,b].
    basis: 0=total, 1=row0, 2=row127, 3=col0, 4=col127,
           5..8=corners (00,0W,H0,HW), 9=const 1."""
    cw = np.zeros((C, 10, K), np.float64)
    scale = 1.0 / (C * H * W)
    for w_net, pads in ((net0_w, (0, 0, 0)), (net1_w, (1, 1, 1)), (net2_w, (2, 1, 1))):
        Kk, _, kd, kh, kw = w_net.shape
        pd, ph, pw = pads
        for i in range(kd):
            clo, chi = max(0, i - pd), min(C - 1, C - 1 + i - pd)
            cmask = np.zeros(C)
            cmask[clo:chi + 1] = 1.0
            for j in range(kh):
                hlo, hhi = max(0, j - ph), min(H - 1, H - 1 + j - ph)
                dropA = 0 if hlo == 1 else (127 if hhi == H - 2 else None)
                for l in range(kw):
                    wlo, whi = max(0, l - pw), min(W - 1, W - 1 + l - pw)
                    dropB = 0 if wlo == 1 else (127 if whi == W - 2 else None)
                    v = np.zeros(10)
                    v[0] = 1.0
                    if dropA == 0: v[1] = -1.0
                    if dropA == 127: v[2] = -1.0
                    if dropB == 0: v[3] = -1.0
                    if dropB == 127: v[4] = -1.0
                    if dropA is not None and dropB is not None:
                        v[{(0, 0): 5, (0, 127): 6, (127, 0): 7, (127, 127): 8}[(dropA, dropB)]] = 1.0
                    for k in range(Kk):
                        cw[:, :, k] += w_net[k, 0, i, j, l] * scale * np.outer(cmask, v)
    btot = (net0_b + net1_b + net2_b).astype(np.float64)
    cw[:, 9, :] += btot[None, :] / C
    return np.ascontiguousarray(cw.astype(np.float32))


def _make_bank(Wt):
    """Wt (co, ci, 3, 3) -> (128, 6, 64): [p=ci(lo)/64+ci(hi), mm, co]."""
    bank = np.zeros((128, 6, 64), np.float32)
    for m, (L, Hh) in enumerate(MM_TAPS):
        bank[:64, m, :] = Wt[:, :, 1 + L[0], 1 + L[1]].T
        if Hh is not None:
            bank[64:, m, :] = Wt[:, :, 1 + Hh[0], 1 + Hh[1]].T
    return bank


# ----------------------------------------------------------------------------
# device program
# ----------------------------------------------------------------------------
_NC_CACHE = {}


def _build_nc(conv_dt):
    import concourse.bacc as bacc
    import concourse.tile as tile
    from concourse import mybir

    f32 = mybir.dt.float32
    if conv_dt == "bf16":
        DT = mybir.dt.bfloat16
    elif conv_dt == "fp32r":
        DT = mybir.dt.float32r
    else:
        DT = f32
    WBDT = mybir.dt.bfloat16 if conv_dt == "bf16" else f32
    MWDT = mybir.dt.float32r if conv_dt == "fp32r" else f32
    Alu = mybir.AluOpType
    Ax = mybir.AxisListType
    Act = mybir.ActivationFunctionType

    nc = bacc.Bacc("TRN2", target_bir_lowering=False, debug=False,
                   enable_asserts=False, num_devices=N)
    xin = nc.dram_tensor("xin", [C, H * WP], DT, kind="ExternalInput")
    wbk = nc.dram_tensor("wbanks", [128, 5, 6 * 64], WBDT, kind="ExternalInput")
    cw2 = nc.dram_tensor("cw2", [C, 10, K], f32, kind="ExternalInput")
    cb = nc.dram_tensor("convb", [C, 1], f32, kind="ExternalInput")
    outT = nc.dram_tensor("out", [C, H, W], f32, kind="ExternalOutput")

    span_elems = WP * (H // NCHUNKS)                   # 8320

    with tile.TileContext(nc) as tc:
        with tc.tile_pool(name="singles", bufs=1) as S, \
             tc.tile_pool(name="stage", bufs=4) as STG, \
             tc.tile_pool(name="cpsum", bufs=4, space="PSUM") as PS, \
             tc.tile_pool(name="spsum", bufs=1, space="PSUM") as PS1:

            XL = S.tile([128, NELEM], DT)
            wb_sb = S.tile([128, 5, 6 * 64], WBDT)
            cw2_sb = S.tile([C, 10, K], f32)
            convb_sb = S.tile([C, 1], f32)
            onesrow = S.tile([128, 128], f32)
            onesall = S.tile([C, 128], f32)
            att_sb = S.tile([128, K], f32)
            attbc = S.tile([128, K], f32)
            M10 = S.tile([C, 10], f32)
            P01 = S.tile([C, 1], f32)
            P23 = S.tile([C, 1], f32)
            PART0 = S.tile([C, 1], f32)
            PART1 = S.tile([C, 1], f32)
            PART2 = S.tile([C, 1], f32)
            PART3 = S.tile([C, 1], f32)
            PARTS = [PART0, PART1, PART2, PART3]
            G = S.tile([C, K], f32)
            mw = S.tile([128, 6, 64], MWDT)
            mwb = S.tile([128, 6, 64], DT, name="mwb") if conv_dt == "bf16" else None
            fold = S.tile([C, 2700], f32)
            fold2 = S.tile([C, 2700], f32)
            actout = S.tile([C, 3300], f32)
            actout2 = S.tile([C, 3300], f32)
            rs128 = S.tile([128, 1], f32)

            wpsum = PS1.tile([128, 512], f32)
            psum_b = PS1.tile([128, K], f32)

            XLv = XL.bitcast(f32) if conv_dt == "fp32r" else XL

            # --- constants / border zeroing (DVE, all tiny) ---
            nc.vector.memset(onesrow, 0.0)
            nc.vector.memset(onesall, 1.0)
            nc.vector.memset(M10[:, 9:10], 1.0)
            # borders: host pre-pads the row gaps; only head/tail need zeroing
            nc.vector.memset(XLv[0:64, 0:132], 0.0)
            nc.vector.memset(XLv[0:64, 132 + H * WP:NELEM], 0.0)
            nc.vector.memset(XLv[64:128, 0:2], 0.0)
            nc.vector.memset(XLv[64:128, 2 + H * WP:NELEM], 0.0)

            # --- small input DMAs (scalar/ACT HWDGE ring) ---
            nc.scalar.dma_start(out=wb_sb, in_=wbk[:, :, :])
            nc.scalar.dma_start(out=cw2_sb, in_=cw2[:, :, :])
            nc.scalar.dma_start(out=convb_sb, in_=cb[:, :])

            # --- x load: contiguous lower chunks from HBM (sync ring), then the
            # row-shifted upper copy via on-chip SBUF->SBUF DMA (no HBM re-read)
            for c in range(NCHUNKS):
                a = span_elems * c
                nc.sync.dma_start(out=XL[0:64, 132 + a: 132 + a + span_elems],
                                  in_=xin[:, a: a + span_elems])
            for c in range(NCHUNKS):
                a = span_elems * c
                nc.sync.dma_start(out=XL[64:128, 2 + a: 2 + a + span_elems],
                                  in_=XL[0:64, 132 + a: 132 + a + span_elems])

            # --- PE warm-up (results discarded; onesrow is all-zero) ---
            for i in range(8):
                nc.tensor.matmul(wpsum[:, 0:128], onesrow, onesrow, start=True, stop=True)

            # --- attention basis sums ---
            # DVE: scalar_tensor_tensor fold (2 streams/cycle) with accum_out;
            # ACT: activation-Identity with accum_out.  (tensor_tensor_reduce
            # is broken on this runtime — do not use.)
            spans = [(0, 5300, "dve", fold), (5300, 3152, "act", actout),
                     (8452, 5248, "dve", fold2), (13700, NELEM - 13700, "act", actout2)]
            # row-0 edge sum can start as soon as chunk 0 lands
            nc.vector.tensor_reduce(out=M10[:, 1:2], in_=XLv[0:64, 132:132 + W],
                                    axis=Ax.X, op=Alu.add)
            for idx, (a, ln, eng, obuf) in enumerate(spans):
                if idx == 2:
                    # chunk-1-dependent small reductions, queued on DVE before
                    # the second big span so the logits pipeline unblocks early
                    nc.vector.tensor_reduce(out=M10[:, 2:3], in_=XLv[0:64, 16642:16642 + W],
                                            axis=Ax.X, op=Alu.add)
                    col0 = XLv[0:64, 132:132 + WP * H].rearrange("p (r w) -> p r w", w=WP)[:, :, 0:1]
                    nc.vector.tensor_reduce(out=M10[:, 3:4], in_=col0, axis=Ax.XY, op=Alu.add)
                    col1 = XLv[0:64, 259:259 + WP * H].rearrange("p (r w) -> p r w", w=WP)[:, :, 0:1]
                    nc.vector.tensor_reduce(out=M10[:, 4:5], in_=col1, axis=Ax.XY, op=Alu.add)
                    # corners {132,259} and {16642,16769} via stride-127 views
                    nc.vector.tensor_copy(
                        out=M10[:, 5:7].rearrange("p (a b) -> p a b", b=1),
                        in_=XLv[0:64, 132:132 + 254].rearrange("p (a b) -> p a b", b=127)[:, :, 0:1])
                    nc.vector.tensor_copy(
                        out=M10[:, 7:9].rearrange("p (a b) -> p a b", b=1),
                        in_=XLv[0:64, 16642:16642 + 254].rearrange("p (a b) -> p a b", b=127)[:, :, 0:1])
                if eng == "dve":
                    h = ln // 2
                    nc.vector.scalar_tensor_tensor(
                        out=obuf[:, :h], in0=XLv[0:64, a:a + h], scalar=1.0,
                        in1=XLv[0:64, a + h:a + ln], op0=Alu.mult, op1=Alu.add,
                        accum_out=PARTS[idx][:, 0:1])
                else:
                    nc.scalar.activation(
                        out=obuf[:, :ln], in_=XLv[0:64, a:a + ln], func=Act.Identity,
                        bias=0.0, scale=1.0, accum_out=PARTS[idx][:, 0:1])

            # fold the 4 span partials into basis column 0 (all on DVE)
            nc.vector.tensor_add(out=P01, in0=PART0, in1=PART1)
            nc.vector.tensor_add(out=P23, in0=PART2, in1=PART3)
            nc.vector.tensor_add(out=M10[:, 0:1], in0=P01, in1=P23)

            # per-channel coefficient contraction: G[c,k] = sum_b M10[c,b]*CW2[c,b,k]
            for k in range(K):
                nc.vector.scalar_tensor_tensor(
                    out=fold[:, 0:10], in0=M10[:, :], scalar=1.0,
                    in1=cw2_sb[:, :, k], op0=Alu.mult, op1=Alu.mult,
                    accum_out=G[:, k:k + 1])

            # keep the PE clock warm into the conv (dummy matmuls on span scratch)
            for i in range(5):
                nc.tensor.matmul(wpsum, onesrow[0:64, :], fold2[:, i * 512:(i + 1) * 512],
                                 start=True, stop=True)

            # logits broadcast to all 128 partitions with one matmul
            nc.tensor.matmul(psum_b, onesall, G, start=True, stop=True)
            # softmax per partition (identical everywhere); logits are tiny, so
            # the max-subtraction is unnecessary
            nc.scalar.activation(out=att_sb, in_=psum_b, func=Act.Exp)
            nc.vector.tensor_reduce(out=rs128, in_=att_sb, axis=Ax.X, op=Alu.add)
            nc.vector.reciprocal(out=rs128, in_=rs128)
            nc.vector.tensor_scalar_mul(out=attbc, in0=att_sb, scalar1=rs128)

            # --- weight mixing: mw = conv_bank + sum_k att_k * bank_k ---
            nc.vector.scalar_tensor_tensor(
                out=mw[:, :, :], in0=wb_sb[:, 0, :].rearrange("p (m c) -> p m c", m=6),
                scalar=attbc[:, 0:1],
                in1=wb_sb[:, 4, :].rearrange("p (m c) -> p m c", m=6),
                op0=Alu.mult, op1=Alu.add)
            for k in range(1, K):
                tgt = mwb if (k == K - 1 and conv_dt == "bf16") else mw
                nc.vector.scalar_tensor_tensor(
                    out=tgt[:, :, :], in0=wb_sb[:, k, :].rearrange("p (m c) -> p m c", m=6),
                    scalar=attbc[:, k:k + 1], in1=mw[:, :, :],
                    op0=Alu.mult, op1=Alu.add)
            lhs_src = mwb if conv_dt == "bf16" else mw

            # --- main conv: 43 PSUM tiles x 6 accumulating matmuls ---
            for ti, r0 in enumerate(range(1, H + 1, ROWS_PER_TILE)):
                nrows = min(ROWS_PER_TILE, H + 1 - r0)
                F = WP * nrows
                pt = PS.tile([64, WP * ROWS_PER_TILE], f32, tag="cps", name=f"cps{ti}")
                pt = pt[:, :F]
                for m in range(6):
                    rhs = XL[:, WP * r0 + MM_OFFS[m] + 1: WP * r0 + MM_OFFS[m] + 1 + F]
                    nc.tensor.matmul(pt, lhs_src[:, m, :], rhs, start=(m == 0), stop=(m == 5))
                st = STG.tile([64, WP * ROWS_PER_TILE], f32, tag="stg", name=f"stg{ti}")
                if ti % 2 == 0:
                    nc.scalar.add(out=st[:, :F], in_=pt, add=convb_sb[:, 0:1])
                else:
                    nc.vector.tensor_scalar_add(out=st[:, :F], in0=pt, scalar1=convb_sb[:, 0:1])
                src = st[:, :F].rearrange("p (r w) -> p r w", w=WP)[:, :, 1:1 + W]
                eng = nc.sync if ti % 2 == 0 else nc.scalar
                eng.dma_start(out=outT[:, r0 - 1:r0 - 1 + nrows, :], in_=src)

    nc.compile()
    return nc


def _get_nc():
    if CONV_DT not in _NC_CACHE:
        _NC_CACHE[CONV_DT] = _build_nc(CONV_DT)
    return _NC_CACHE[CONV_DT]


def _prep_inputs(x, weight, conv_w, conv_b, net0_w, net0_b, net1_w, net1_b,
                 net2_w, net2_b):
    cw2 = _make_cw2(np.asarray(net0_w, np.float32), np.asarray(net0_b, np.float32),
                    np.asarray(net1_w, np.float32), np.asarray(net1_b, np.float32),
                    np.asarray(net2_w, np.float32), np.asarray(net2_b, np.float32))
    banks = np.stack([_make_bank(np.asarray(weight, np.float32)[k]) for k in range(K)]
                     + [_make_bank(np.asarray(conv_w, np.float32))])  # (5,128,6,64)
    banks = np.ascontiguousarray(banks.reshape(5, 128, 6 * 64).transpose(1, 0, 2))
    convb = np.ascontiguousarray(np.asarray(conv_b, np.float32).reshape(C, 1))
    x = np.asarray(x, np.float32)
    xp = np.zeros((N, C, H, WP), np.float32)
    xp[:, :, :, :W] = x
    if CONV_DT == "bf16":
        import ml_dtypes
        xs = xp.astype(ml_dtypes.bfloat16)
        banks = banks.astype(ml_dtypes.bfloat16)
    else:
        xs = xp
    in_maps = []
    for n in range(N):
        in_maps.append({
            "xin": np.ascontiguousarray(xs[n].reshape(C, H * WP)),
            "wbanks": banks,
            "cw2": cw2,
            "convb": convb,
        })
    return in_maps


def _run(inputs, trace=False, **kw):
    from concourse.bass_utils import run_bass_kernel_spmd
    nc = _get_nc()
    in_maps = _prep_inputs(**inputs)
    return run_bass_kernel_spmd(nc, in_maps, core_ids=list(range(N)), trace=trace, **kw)


def kernel(**inputs):
    res = _run(inputs)
    out = np.stack([res.results[n]["out"] for n in range(N)]).astype(np.float32)
    return out
